# revision 9
# baseline (speedup 1.0000x reference)
"""Trainium2 Bass kernel for nn_CHIVE_53111565583018 (clockwork-RNN CHIVE).

The model is a strictly sequential scan (T=131072 encoder steps, K~65536
decoder steps) with tiny (<=32-dim) state, but every chain is strongly
contracting (tanh/gelu + clockwork holds forget initial conditions fast).
We run *chunked* scans: each core processes its time/k range as ~256 chunks
held in the matmul free dimension, advancing all chunks in lockstep, with
2-3 "sweeps" where sweep s re-seeds chunk i with chunk i-1's final state
from sweep s-1.  Boundary error after S sweeps ~ contraction(L)^S, validated
offline (final output rel-err <= ~1e-3).

Per-core phases (8 cores SPMD; core c owns 1/8 of the decoder k-range and an
encoder time window covering it):
  E: encoder frnn/phrnn chains, 64-step chunks x 265 lanes, 2 sweeps;
     records hf+hp per step (via a selection matmul).
  G: repack history to quarter-packed layout (DMA) + indirect_copy gather
     of hf+hp at syllable positions (k-space).
  C: sylrnn hs chain, dense in k-space, 32-step chunks x 4 quarter-chains
     packed on partitions, 2 sweeps; records ys.
  Z: decoder parallel math: mean/logvar/z/hs_new/hp_new/UV projections and
     the cond fill-forward (tensor_tensor_scan).
  D: hff/hfc chain, 32-step chunks, 3 sweeps.
Final outputs (hff[1], hfc[12], kl) assembled on host from core 7's exports.
"""
import os
import sys
import numpy as np
from contextlib import ExitStack

sys.path.insert(0, "/opt/trn_rl_repo")

import concourse.bass as bass
import concourse.bacc as bacc
import concourse.tile as tile
from concourse import mybir

# ---------------- constants (problem-specific, hardcoded) ----------------
T = 131072
H = 32
NCORES = 8

LE = 64            # encoder chunk length
NRE = 264          # encoder real lanes/core
ELANES = NRE + 1   # + prefix lane = 265
ECOV = NRE * LE    # 16896 encoder steps covered per core
EPRE = 64          # leading prefix columns in encoder buffers
EBUFM = EPRE + ECOV           # mask buffer cols
EBUFX = EPRE + ECOV + 64      # xfp buffer cols (tail pad for tick 64 reads)

LK = 32            # k-space chunk length (C and D chains)
QREAL = 2056       # real k slots per quarter
EXT = 24           # leading real-k extension per quarter
QW = EXT + QREAL   # 2080 recorded k slots per quarter
QWB = 32 + QW      # 2112: + prefix-lane slots (C chain / gather)
NRQ = QW // LK     # 65 real lanes per quarter chain
QLANES = NRQ + 1   # 66
DPAD = LK - EXT    # 8 zero cols ahead of UVD
UVDW = DPAD + QW + EXT  # 2112
XBUF2 = EBUFX // 2 + 32   # 2-block xfp layout cols
MBUF2 = EBUFM // 2        # 2-block mask layout cols

E_SWEEPS = 2
C_SWEEPS = 2
D_SWEEPS = 3

DT = mybir.dt.float32
I8 = mybir.dt.int8
U16 = mybir.dt.uint16
AF = mybir.ActivationFunctionType

TRACE = bool(int(os.environ.get("KERNEL_TRACE", "0")))
DEBUG_OUTS = bool(int(os.environ.get("KERNEL_DEBUG_OUTS", "0")))
LAST = {}  # exec info for the test harness


def _cols(ap2d, start, stride, count):
    return ap2d[:, start: start + (count - 1) * stride + 1: stride]


def _blkdiag4(w):
    n, m = w.shape
    out = np.zeros((4 * n, 4 * m), np.float32)
    for q in range(4):
        out[q * n:(q + 1) * n, q * m:(q + 1) * m] = w
    return out


def _tile4(v):
    return np.tile(np.asarray(v, np.float32), 4)


# ---------------- host-side preprocessing ----------------

def _prep(inputs):
    p = inputs["params"]

    def P(name):
        d = p[name]
        return (np.asarray(d["Wx"], np.float32), np.asarray(d["bx"], np.float32),
                np.asarray(d["Wh"], np.float32), np.asarray(d["bh"], np.float32))

    Wxf0, bxf0, Whf0, bhf0 = P("frnn0")
    Wxf1, bxf1, Whf1, bhf1 = P("frnn1")
    Wxp0, bxp0, Whp0, bhp0 = P("phrnn0")
    Wxp1, bxp1, Whp1, bhp1 = P("phrnn1")
    Wxs, bxs, Wsh, bhs = P("sylrnn")
    Wxd, bxd, _Whd, bhd = P("phrnn_decd")
    Wxff, bxff, Whff, bhff = P("frnn_f")
    Wxfc, bxfc, Whfc, bhfc = P("frnn_c")
    Wm, bm = [np.asarray(x, np.float32) for x in p["bn_mean"]]
    Wlv, blv = [np.asarray(x, np.float32) for x in p["bn_logvar"]]

    xf = np.asarray(inputs["frnn_seq"], np.float32)
    xp = np.asarray(inputs["phrnn_seq"], np.float32)
    xs = np.asarray(inputs["sylrnn_seq"], np.float32)
    eps = np.asarray(inputs["eps"], np.float32)
    cf = np.asarray(inputs["frnn_clock"])
    cp = np.asarray(inputs["phrnn_clock"])
    sf = np.asarray(inputs["sample_freq"])
    dc = np.asarray(inputs["dec_clock"])
    dcc = np.asarray(inputs["dec_clock_c"])

    ts = np.arange(T)
    mf = ((ts % cf) == 0).astype(np.int8)
    mp = ((ts % cp) == 0).astype(np.int8)
    idx = np.nonzero(sf == 1)[0].astype(np.int64)
    K = len(idx)
    i_arr = np.arange(K)
    mcd = ((i_arr % dc[:K]) == 0).astype(np.float32)
    mcc = ((i_arr % dcc[:K]) == 0).astype(np.int8)
    cond = np.zeros(K, np.float32)
    cond[1:] = (sf[:K - 1] == 1).astype(np.float32)

    kb = [round(c * K / NCORES) for c in range(NCORES + 1)]

    # ---- weight blocks (shared) ----
    # W1full: out rows 0:64 = A-state pre-act recurrent part,
    #         rows 64:128 = B pre-act (x-part from A-state + recurrent part)
    wW1 = np.zeros((128, 128), np.float32)
    wW1[0:32, 0:32] = Whf0
    wW1[32:64, 32:64] = Whp0
    wW1[0:32, 64:96] = Wxf1
    wW1[32:64, 96:128] = Wxp1
    wW1[64:96, 64:96] = Whf1
    wW1[96:128, 96:128] = Whp1
    wWX1 = np.zeros((21, 128), np.float32)
    wWX1[0:13, 0:32] = Wxf0
    wWX1[13:20, 32:64] = Wxp0
    wWX1[20, 0:32] = bxf0 + bhf0
    wWX1[20, 32:64] = bxp0 + bhp0
    wWX1[20, 64:96] = bxf1 + bhf1
    wWX1[20, 96:128] = bxp1 + bhp1
    wWX = np.zeros((128, 128), np.float32)   # replicated per 64-row block
    for b in range(2):
        wWX[64 * b:64 * b + 21] = wWX1
    wSel = np.zeros((128, 32), np.float32)   # hf + hp fold for the record
    for i in range(H):
        wSel[64 + i, i] = 1.0
        wSel[96 + i, i] = 1.0

    Wu = np.concatenate([Wxff, Wxfc], axis=1)          # (32,13)
    bu = np.concatenate([bxff + bhff, bxfc + bhfc])    # (13,)
    DD = np.zeros((13, 13), np.float32)
    DD[0, 0] = Whff[0, 0]
    DD[1:, 1:] = Whfc

    weights = {
        "wW1": wW1, "wWX": wWX, "wSel": wSel,
        "wC1": _blkdiag4(Wsh), "wC2": _blkdiag4(Wxs),
        "wMm": _blkdiag4(Wm), "wMlv": _blkdiag4(Wlv),
        "wZ1": _blkdiag4(Wxs), "wZ2": _blkdiag4(Wxd),
        "wU": _blkdiag4(Wu), "wD": _blkdiag4(DD),
        "wEye": np.eye(52, dtype=np.float32),
    }
    bias = np.zeros((128, 8), np.float32)
    bias[:, 0] = _tile4(bxs + bhs)        # C chain / hs_new
    bias[:, 1] = _tile4(bm)               # mean
    bias[:, 2] = _tile4(0.5 * blv)        # 0.5*logvar bias for exp
    bias[:, 3] = _tile4(bxd + bhd)        # hp_new
    bias[0:52, 4] = _tile4(bu)            # UV bias
    weights["bias"] = bias

    # ---- per-core geometry ----
    geo = []
    for c in range(NCORES):
        k0, k1 = kb[c], kb[c + 1]
        assert k1 - k0 <= 4 * QREAL
        qs = [k0 + q * QREAL for q in range(4)]
        kwin0 = max(0, k0 - EXT - 32)
        start = 0 if c == 0 else int(idx[kwin0])
        assert int(idx[k1 - 1]) - start < ECOV, \
            f"core {c}: encoder window too small"
        geo.append({"k0": k0, "k1": k1, "qs": qs, "start": start})

    # uniform per-quarter record-window lane offsets (lane-aligned, shared
    # across cores so the program is SPMD-uniform)
    lq = []
    NLQ = 0
    for q in range(4):
        los, his = [], []
        for c in range(NCORES):
            g = geo[c]
            kq0 = max(0, g["qs"][q] - EXT - 32)
            kqL = min(g["k1"], g["qs"][q] + QREAL) - 1
            los.append(int(idx[kq0]) - g["start"])
            his.append(int(idx[kqL]) - g["start"])
        L_q = max(0, min(los) // LE)
        lq.append(L_q)
        NLQ = max(NLQ, -(-(max(his) + 1 - LE * L_q) // LE))
    NLQ += 1  # margin lane
    assert max(lq) + NLQ <= NRE, f"record window overflow {lq} {NLQ}"
    EQ = NLQ * LE

    # ---- per-core data ----
    cores = []
    for c in range(NCORES):
        g = geo[c]
        k0, k1, qs, start = g["k0"], g["k1"], g["qs"], g["start"]

        tloc = np.arange(EBUFX) - EPRE + start
        ok = (tloc >= 0) & (tloc < T)
        tc_ = np.clip(tloc, 0, T - 1)
        xfp_flat = np.zeros((21, EBUFX), np.float32)
        xfp_flat[0:13] = np.where(ok, xf[tc_].T, 0.0)
        xfp_flat[13:20] = np.where(ok, xp[tc_].T, 0.0)
        xfp_flat[20] = 1.0
        # 2-block layout: flat col j -> block j%2, col j//2
        u_xfp = np.zeros((128, XBUF2), np.float32)
        for b in range(2):
            cols = np.arange(b, EBUFX, 2)
            u_xfp[64 * b:64 * b + 21, 0:len(cols)] = xfp_flat[:, cols]
        msk_flat = np.zeros((64, EBUFM), np.int8)
        okm = ok[:EBUFM]
        tcm = tc_[:EBUFM]
        msk_flat[0:32] = np.where(okm, mf[tcm], 0)
        msk_flat[32:64] = np.where(okm, mp[tcm], 0)
        # 2-block layout (even/odd steps), A rows 0:64 and B rows 64:128
        u_mskE = np.zeros((128, MBUF2), np.int8)
        u_mskO = np.zeros((128, MBUF2), np.int8)
        u_mskE[0:64] = msk_flat[:, 0::2]
        u_mskE[64:128] = u_mskE[0:64]
        u_mskO[0:64] = msk_flat[:, 1::2]
        u_mskO[64:128] = u_mskO[0:64]

        u_gidx = np.zeros((128, QWB // 16), np.uint16)
        u_xsK = np.zeros((128, QWB), np.float32)
        u_eps = np.zeros((128, QW), np.float32)
        u_mcd = np.zeros((128, QW), np.float32)
        u_cnd = np.zeros((52, QW), np.float32)
        u_mcc = np.zeros((52, UVDW), np.int8)
        for q in range(4):
            # gather/C-chain col j (0..QWB) <-> k = qs[q] - EXT - 32 + j
            kk = qs[q] - EXT - 32 + np.arange(QWB)
            live = (kk >= 0) & (kk < k1)
            kkc = np.clip(kk, 0, K - 1)
            tk = np.where(live, idx[kkc] - start - LE * lq[q], 0).astype(np.int64)
            assert tk.min() >= 0 and tk.max() < EQ, f"core {c} q{q} idx range"
            w16 = tk.reshape(QWB // 16, 16).T.astype(np.uint16)
            u_gidx[32 * q:32 * q + 16] = w16
            u_gidx[32 * q + 16:32 * q + 32] = w16
            u_xsK[32 * q:32 * q + 32] = np.where(live, xs[idx[kkc]].T, 0.0)
            # ys/Z col j2 (0..QW) <-> k = qs[q] - EXT + j2
            kk2 = kk[32:]
            live2 = live[32:]
            kkc2 = kkc[32:]
            u_eps[32 * q:32 * q + 32] = np.where(live2, eps[kkc2].T, 0.0)
            u_mcd[32 * q:32 * q + 32] = np.where(live2, mcd[kkc2], 0.0)
            u_cnd[13 * q:13 * q + 13] = np.where(live2, cond[kkc2], 0.0)
            # UVD col j' <-> k = qs[q] + j' - 32
            kk3 = qs[q] + np.arange(UVDW) - 32
            live3 = (kk3 >= 0) & (kk3 < k1) & (np.arange(UVDW) >= DPAD)
            kkc3 = np.clip(kk3, 0, K - 1)
            u_mcc[13 * q, :] = np.where(live3, 1, 0)
            u_mcc[13 * q + 1:13 * q + 13, :] = np.where(live3, mcc[kkc3], 0)

        cores.append({
            "u_xfp": u_xfp, "u_mskE": u_mskE, "u_mskO": u_mskO,
            "u_gidx": u_gidx,
            "u_xsK": u_xsK, "u_eps": u_eps, "u_mcd": u_mcd,
            "u_cnd": u_cnd, "u_mcc": u_mcc,
        })

    return {"K": K, "kb": kb, "bm": bm, "blv": blv, "weights": weights,
            "geo": geo, "cores": cores, "lq": lq, "NLQ": NLQ, "EQ": EQ}


# ---------------- bass program ----------------

def build_program(lq, NLQ, EQ):
    nc = bacc.Bacc()

    din = {}
    for name, shape, dt in [
        ("u_xfp", (128, XBUF2), DT), ("u_mskE", (128, MBUF2), I8),
        ("u_mskO", (128, MBUF2), I8),
        ("u_gidx", (128, QWB // 16), U16),
        ("u_xsK", (128, QWB), DT), ("u_eps", (128, QW), DT),
        ("u_mcd", (128, QW), DT), ("u_cnd", (52, QW), DT),
        ("u_mcc", (52, UVDW), I8),
        ("wW1", (128, 128), DT), ("wWX", (128, 128), DT),
        ("wSel", (128, 32), DT),
        ("wC1", (128, 128), DT), ("wC2", (128, 128), DT),
        ("wMm", (128, 128), DT), ("wMlv", (128, 128), DT),
        ("wZ1", (128, 128), DT), ("wZ2", (128, 128), DT),
        ("wU", (128, 52), DT), ("wD", (52, 52), DT),
        ("wEye", (52, 52), DT), ("bias", (128, 8), DT),
    ]:
        din[name] = nc.declare_dram_parameter(name, list(shape), dt,
                                              isOutput=False)
    dout = {}
    outs = [("o_SD", (52, QLANES), DT), ("o_mean", (32, QW), DT),
            ("o_lv", (32, QW), DT)]
    if DEBUG_OUTS:
        outs += [("o_hq", (128, EQ), DT), ("o_ys", (128, QW), DT),
                 ("o_uvd", (52, UVDW), DT), ("o_resk", (128, QWB), DT)]
    for name, shape, dt in outs:
        dout[name] = nc.declare_dram_parameter(name, list(shape), dt,
                                               isOutput=True)

    with tile.TileContext(nc) as tc, ExitStack() as ctx:
        pc = ctx.enter_context(tc.tile_pool(name="const", bufs=1))
        po = ctx.enter_context(tc.tile_pool(name="outer", bufs=1))

        w = {}
        for name in ["wW1", "wWX", "wSel", "wC1", "wC2", "wMm", "wMlv",
                     "wZ1", "wZ2", "wU", "wD", "wEye", "bias", "u_gidx",
                     "u_xsK", "u_eps", "u_mcd", "u_cnd", "u_mcc"]:
            t = pc.tile(list(din[name].shape), din[name].dtype, tag=name)
            nc.gpsimd.dma_start(t[:], din[name][:])
            w[name] = t
        bias = w["bias"]

        # engine warm-ups: let each engine observe the DMA sems on a cheap op
        wrm = pc.tile([128, 8], DT)
        nc.vector.tensor_copy(wrm[0:52, 0:1], w["u_cnd"][:, 0:1])
        nc.vector.tensor_copy(wrm[:, 1:2], w["u_mcd"][:, 0:1])
        nc.vector.tensor_copy(wrm[:, 2:3], w["u_eps"][:, 0:1])
        nc.vector.tensor_copy(wrm[:, 3:4], w["u_xsK"][:, 0:1])
        wrm8 = pc.tile([128, 2], I8)
        nc.vector.tensor_copy(wrm8[0:52, 0:1], w["u_mcc"][:, 0:1])
        wrm16 = pc.tile([128, 1], U16)
        nc.gpsimd.tensor_copy(wrm16[:], w["u_gidx"][:, 0:1])

        # persistent k-space tiles
        ysK = po.tile([128, QW], DT)
        RESK = po.tile([128, QWB], DT)
        UV52 = po.tile([52, QW], DT)
        UVD = po.tile([52, UVDW], DT)

        # ---------------- phase E ----------------
        S128 = po.tile([128, ELANES], DT)   # rows 0:64 A, rows 64:128 B
        TMP = po.tile([128, ELANES], DT)    # rows 64:128 used
        SHT = po.tile([128, ELANES], DT)
        nc.vector.memset(S128[:], 0.0)
        nc.vector.memset(TMP[:], 0.0)

        with tc.tile_pool(name="hq", bufs=1) as p_hq:
            HQR = p_hq.tile([128, EQ], DT)   # quarter-windowed hf+hp history
            with tc.tile_pool(name="ein", bufs=1) as p_ein, \
                 tc.tile_pool(name="est", bufs=3) as p_est, \
                 tc.tile_pool(name="e_ps", bufs=3, space="PSUM") as p_epp, \
                 tc.tile_pool(name="e_psr", bufs=3, space="PSUM") as p_epr:
                xfp = p_ein.tile([128, XBUF2], DT)
                mskE = p_ein.tile([128, MBUF2], I8)
                mskO = p_ein.tile([128, MBUF2], I8)
                nc.gpsimd.dma_start(xfp[:], din["u_xfp"][:])
                nc.gpsimd.dma_start(mskE[:], din["u_mskE"][:])
                nc.gpsimd.dma_start(mskO[:], din["u_mskO"][:])
                nc.vector.tensor_copy(wrm8[:, 1:2], mskE[:, 0:1])
                nc.vector.tensor_copy(wrm8[0:64, 0:1], mskO[0:64, 0:1])

                def mska(step):
                    t = mskE if step % 2 == 0 else mskO
                    return _cols(t[0:64, :], step // 2, LK, ELANES)

                def mskb(step):
                    t = mskE if step % 2 == 0 else mskO
                    return _cols(t[64:128, :], step // 2, LK, ELANES)

                for sweep in range(E_SWEEPS):
                    if sweep:
                        nc.vector.tensor_copy(SHT[:], S128[:])
                        nc.vector.tensor_copy(S128[:, 1:ELANES],
                                              SHT[:, 0:ELANES - 1])
                        nc.vector.memset(S128[:, 0:1], 0.0)
                        nc.vector.tensor_copy(TMP[64:128, :], S128[64:128, :])
                    for tk in range(LE + 1):
                        a_on = tk < LE
                        b_on = tk >= 1
                        P = p_epp.tile([128, ELANES], DT)
                        b2 = 64 * (tk % 2)
                        nc.tensor.matmul(P[:], w["wWX"][b2:b2 + 21, :],
                                         _cols(xfp[b2:b2 + 21, :], tk // 2,
                                               32, ELANES),
                                         start=True, stop=False)
                        nc.tensor.matmul(P[:], w["wW1"][:], S128[:],
                                         start=False, stop=True)
                        T1 = p_est.tile([128, ELANES], DT)
                        nc.scalar.activation(T1[:], P[:], AF.Tanh)
                        if a_on:
                            nc.vector.copy_predicated(
                                S128[0:64, :], mska(tk), T1[0:64, :])
                        if b_on:
                            sB = tk - 1
                            nc.vector.copy_predicated(
                                TMP[64:128, :], mskb(sB), T1[64:128, :])
                            nc.scalar.activation(S128[64:128, :],
                                                 TMP[64:128, :], AF.Gelu)
                            nc.vector.tensor_copy(TMP[64:128, :],
                                                  S128[64:128, :])
                            if sweep == E_SWEEPS - 1:
                                PR = p_epr.tile([32, ELANES], DT)
                                nc.tensor.matmul(PR[:], w["wSel"][:],
                                                 S128[:], start=True,
                                                 stop=True)
                                for q in range(4):
                                    nc.vector.tensor_copy(
                                        _cols(HQR[32 * q:32 * q + 32, :],
                                              sB, LE, NLQ),
                                        PR[:, lq[q] + 1:lq[q] + NLQ + 1])
            if DEBUG_OUTS:
                nc.sync.dma_start(dout["o_hq"][:], HQR[:])

            # ---------------- phase G ----------------
            with tc.tile_pool(name="gat", bufs=1) as p_g:
                HFPK = p_g.tile([128, QWB], DT)
                nc.gpsimd.tensor_copy(wrm[:, 4:5], HQR[:, 0:1])
                NIC = 352  # indirect_copy output-width ISA cap is ~448
                for o in range(0, QWB, NIC):
                    nc.gpsimd.indirect_copy(
                        HFPK[:, o:o + NIC], HQR[:],
                        w["u_gidx"][:, o // 16:(o + NIC) // 16], True)
                nc.vector.tensor_add(RESK[:], HFPK[:], w["u_xsK"][:])
        if DEBUG_OUTS:
            nc.sync.dma_start(dout["o_resk"][:], RESK[:])

        # ---------------- phase C ----------------
        S_C = po.tile([128, QLANES], DT)
        nc.vector.memset(S_C[:], 0.0)
        SHC = po.tile([128, QLANES], DT)
        with tc.tile_pool(name="cst", bufs=3) as p_cst, \
             tc.tile_pool(name="c_ps", bufs=3, space="PSUM") as p_cpp:
            for sweep in range(C_SWEEPS):
                if sweep:
                    nc.vector.tensor_copy(SHC[:], S_C[:])
                    nc.vector.tensor_copy(S_C[:, 1:QLANES],
                                          SHC[:, 0:QLANES - 1])
                    nc.vector.memset(S_C[:, 0:1], 0.0)
                for s in range(LK):
                    P = p_cpp.tile([128, QLANES], DT)
                    nc.tensor.matmul(P[:], w["wC2"][:],
                                     _cols(RESK[:], s, LK, QLANES),
                                     start=True, stop=False)
                    nc.tensor.matmul(P[:], w["wC1"][:], S_C[:],
                                     start=False, stop=True)
                    TC = p_cst.tile([128, QLANES], DT)
                    nc.scalar.activation(TC[:], P[:], AF.Tanh,
                                         bias=bias[:, 0:1])
                    nc.scalar.activation(S_C[:], TC[:], AF.Gelu)
                    if sweep == C_SWEEPS - 1:
                        nc.vector.tensor_copy(_cols(ysK[:], s, LK, NRQ),
                                              S_C[:, 1:QLANES])
        if DEBUG_OUTS:
            nc.sync.dma_start(dout["o_ys"][:], ysK[:])

        # ---------------- phase Z ----------------
        SL = [(i, min(512, QW - i)) for i in range(0, QW, 512)]
        with tc.tile_pool(name="zbuf", bufs=1) as p_z, \
             tc.tile_pool(name="z_ps", bufs=4, space="PSUM") as p_zpp:
            MEAN = p_z.tile([128, QW], DT)
            LV = p_z.tile([128, QW], DT)
            ET = p_z.tile([128, QW], DT)
            PZ = p_z.tile([128, QW], DT)
            ZZ = p_z.tile([128, QW], DT)
            TS = p_z.tile([128, QW], DT)
            GS = p_z.tile([128, QW], DT)
            HSN = p_z.tile([128, QW], DT)
            HPN = p_z.tile([128, QW], DT)

            for dst, wm in ((MEAN, "wMm"), (LV, "wMlv")):
                for o, n in SL:
                    PM = p_zpp.tile([128, 512], DT)
                    nc.tensor.matmul(PM[:, 0:n], w[wm][:], ysK[:, o:o + n],
                                     start=True, stop=True)
                    nc.vector.tensor_copy(dst[:, o:o + n], PM[:, 0:n])
            nc.sync.dma_start(dout["o_mean"][:], MEAN[96:128, :])
            nc.sync.dma_start(dout["o_lv"][:], LV[96:128, :])
            nc.scalar.activation(ET[:], LV[:], AF.Exp,
                                 bias=bias[:, 2:3], scale=0.5)
            nc.vector.tensor_mul(PZ[:], ET[:], w["u_eps"][:])
            nc.vector.scalar_tensor_tensor(
                ZZ[:], MEAN[:], bias[:, 1:2], PZ[:],
                op0=mybir.AluOpType.add, op1=mybir.AluOpType.add)
            for o, n in SL:
                PM = p_zpp.tile([128, 512], DT)
                nc.tensor.matmul(PM[:, 0:n], w["wZ1"][:], ZZ[:, o:o + n],
                                 start=True, stop=True)
                nc.scalar.activation(TS[:, o:o + n], PM[:, 0:n], AF.Tanh,
                                     bias=bias[:, 0:1])
            nc.scalar.activation(GS[:], TS[:], AF.Gelu)
            nc.vector.tensor_mul(HSN[:], GS[:], w["u_mcd"][:])
            for o, n in SL:
                PM = p_zpp.tile([128, 512], DT)
                nc.tensor.matmul(PM[:, 0:n], w["wZ2"][:], HSN[:, o:o + n],
                                 start=True, stop=True)
                nc.scalar.activation(TS[:, o:o + n], PM[:, 0:n], AF.Tanh,
                                     bias=bias[:, 3:4])
            nc.scalar.activation(GS[:], TS[:], AF.Gelu)
            nc.vector.tensor_mul(HPN[:], GS[:], w["u_mcd"][:])
            for o, n in SL:
                PU = p_zpp.tile([128, 512], DT)
                nc.tensor.matmul(PU[0:52, 0:n], w["wU"][:], HPN[:, o:o + n],
                                 start=True, stop=True)
                nc.scalar.activation(UV52[:, o:o + n], PU[0:52, 0:n],
                                     AF.Identity, bias=bias[0:52, 4:5])
            CM = p_z.tile([52, QW], DT)
            CUV = p_z.tile([52, QW], DT)
            nc.vector.tensor_scalar(CM[:], w["u_cnd"][:], -1.0, 1.0,
                                    op0=mybir.AluOpType.mult,
                                    op1=mybir.AluOpType.add)
            nc.vector.tensor_mul(CUV[:], UV52[:], w["u_cnd"][:])
            nc.vector.memset(UVD[:, 0:DPAD], 0.0)
            nc.vector.memset(UVD[:, DPAD + QW:], 0.0)
            nc.vector.tensor_tensor_scan(
                UVD[:, DPAD:DPAD + QW], CM[:], CUV[:], 0.0,
                op0=mybir.AluOpType.mult, op1=mybir.AluOpType.add)
        if DEBUG_OUTS:
            nc.sync.dma_start(dout["o_uvd"][:], UVD[:])

        # ---------------- phase D ----------------
        S_D = po.tile([52, QLANES], DT)
        nc.vector.memset(S_D[:], 0.0)
        SHD = po.tile([52, QLANES], DT)
        with tc.tile_pool(name="dst", bufs=3) as p_dst, \
             tc.tile_pool(name="d_ps", bufs=3, space="PSUM") as p_dpp:
            for sweep in range(D_SWEEPS):
                if sweep:
                    nc.vector.tensor_copy(SHD[:], S_D[:])
                    nc.vector.tensor_copy(S_D[:, 1:QLANES],
                                          SHD[:, 0:QLANES - 1])
                    nc.vector.memset(S_D[:, 0:1], 0.0)
                for s in range(LK):
                    P = p_dpp.tile([52, QLANES], DT)
                    nc.tensor.matmul(P[:], w["wEye"][:],
                                     _cols(UVD[:], s, LK, QLANES),
                                     start=True, stop=False)
                    nc.tensor.matmul(P[:], w["wD"][:], S_D[:],
                                     start=False, stop=True)
                    TD = p_dst.tile([52, QLANES], DT)
                    nc.scalar.activation(TD[:], P[:], AF.Tanh)
                    nc.vector.copy_predicated(
                        S_D[:], _cols(w["u_mcc"][:], s, LK, QLANES), TD[:])
        nc.sync.dma_start(dout["o_SD"][:], S_D[:])

    nc.finalize()
    return nc


# ---------------- entry point ----------------

def kernel(**inputs):
    host = _prep(inputs)
    nc = build_program(host["lq"], host["NLQ"], host["EQ"])

    in_maps = []
    for c in range(NCORES):
        m = {k: np.ascontiguousarray(v) for k, v in host["weights"].items()}
        for name, v in host["cores"][c].items():
            m[name] = np.ascontiguousarray(v)
        in_maps.append(m)

    from concourse.bass_utils import run_bass_kernel_spmd
    res = run_bass_kernel_spmd(nc, in_maps, list(range(NCORES)),
                               trace=TRACE)
    LAST["exec_time_ns"] = res.exec_time_ns
    LAST["results"] = res.results
    LAST["insts"] = res.instructions_and_trace

    K = host["K"]
    g7 = host["geo"][NCORES - 1]
    qs3 = g7["qs"][3]
    r = res.results[NCORES - 1]
    lane = (K - 1 - qs3) // LK + 1
    SD = np.asarray(r["o_SD"])
    hff = np.array([SD[39, lane]], np.float32)
    hfc = SD[40:52, lane].astype(np.float32)
    j = K - 1 - qs3 + EXT
    mean = np.asarray(r["o_mean"])[:, j] + host["bm"]
    lv = np.asarray(r["o_lv"])[:, j] + host["blv"]
    kl = np.float32(-0.5 * np.sum(1.0 + lv - mean * mean - np.exp(lv)))
    return hff, hfc, kl


# revision 11
# speedup vs baseline: 1.4564x; 1.4564x over previous
"""Trainium2 Bass kernel for nn_CHIVE_53111565583018 (clockwork-RNN CHIVE).

The model is a strictly sequential scan (T=131072 encoder steps, K~65536
decoder steps) with tiny (<=32-dim) state, but every chain is strongly
contracting (tanh/gelu + clockwork holds forget initial conditions fast).
We run *chunked* scans: each core processes its time/k range as ~256 chunks
held in the matmul free dimension, advancing all chunks in lockstep, with
2-3 "sweeps" where sweep s re-seeds chunk i with chunk i-1's final state
from sweep s-1.  Boundary error after S sweeps ~ contraction(L)^S, validated
offline (final output rel-err <= ~1e-3).

Per-core phases (8 cores SPMD; core c owns 1/8 of the decoder k-range and an
encoder time window covering it):
  E: encoder frnn/phrnn chains, 64-step chunks x 265 lanes, 2 sweeps;
     records hf+hp per step (via a selection matmul).
  G: repack history to quarter-packed layout (DMA) + indirect_copy gather
     of hf+hp at syllable positions (k-space).
  C: sylrnn hs chain, dense in k-space, 32-step chunks x 4 quarter-chains
     packed on partitions, 2 sweeps; records ys.
  Z: decoder parallel math: mean/logvar/z/hs_new/hp_new/UV projections and
     the cond fill-forward (tensor_tensor_scan).
  D: hff/hfc chain, 32-step chunks, 3 sweeps.
Final outputs (hff[1], hfc[12], kl) assembled on host from core 7's exports.
"""
import os
import sys
import numpy as np
from contextlib import ExitStack

sys.path.insert(0, "/opt/trn_rl_repo")

import concourse.bass as bass
import concourse.bacc as bacc
import concourse.tile as tile
from concourse import mybir

# ---------------- constants (problem-specific, hardcoded) ----------------
T = 131072
H = 32
NCORES = 8

LE = 64            # encoder chunk length
NRE = 264          # encoder real lanes/core
ELANES = NRE + 1   # + prefix lane = 265
ECOV = NRE * LE    # 16896 encoder steps covered per core
EPRE = 64          # leading prefix columns in encoder buffers
EBUFM = EPRE + ECOV           # mask buffer cols
EBUFX = EPRE + ECOV + 64      # xfp buffer cols (tail pad for tick 64 reads)

LK = 32            # k-space chunk length (C and D chains)
QREAL = 2056       # real k slots per quarter
EXT = 24           # leading real-k extension per quarter
QW = EXT + QREAL   # 2080 recorded k slots per quarter
QWB = 32 + QW      # 2112: + prefix-lane slots (C chain / gather)
NRQ = QW // LK     # 65 real lanes per quarter chain
QLANES = NRQ + 1   # 66
DPAD = LK - EXT    # 8 zero cols ahead of UVD
UVDW = DPAD + QW + EXT  # 2112
XBUF2 = EBUFX // 2 + 32   # 2-block xfp layout cols
MBUF2 = EBUFM // 2        # 2-block mask layout cols

E_SWEEPS = 1
C_SWEEPS = 1
D_SWEEPS = 3

DT = mybir.dt.float32
I8 = mybir.dt.int8
U16 = mybir.dt.uint16
AF = mybir.ActivationFunctionType

TRACE = bool(int(os.environ.get("KERNEL_TRACE", "0")))
DEBUG_OUTS = bool(int(os.environ.get("KERNEL_DEBUG_OUTS", "0")))
LAST = {}  # exec info for the test harness


def _cols(ap2d, start, stride, count):
    return ap2d[:, start: start + (count - 1) * stride + 1: stride]


def _blkdiag4(w):
    n, m = w.shape
    out = np.zeros((4 * n, 4 * m), np.float32)
    for q in range(4):
        out[q * n:(q + 1) * n, q * m:(q + 1) * m] = w
    return out


def _tile4(v):
    return np.tile(np.asarray(v, np.float32), 4)


# ---------------- host-side preprocessing ----------------

def _prep(inputs):
    p = inputs["params"]

    def P(name):
        d = p[name]
        return (np.asarray(d["Wx"], np.float32), np.asarray(d["bx"], np.float32),
                np.asarray(d["Wh"], np.float32), np.asarray(d["bh"], np.float32))

    Wxf0, bxf0, Whf0, bhf0 = P("frnn0")
    Wxf1, bxf1, Whf1, bhf1 = P("frnn1")
    Wxp0, bxp0, Whp0, bhp0 = P("phrnn0")
    Wxp1, bxp1, Whp1, bhp1 = P("phrnn1")
    Wxs, bxs, Wsh, bhs = P("sylrnn")
    Wxd, bxd, _Whd, bhd = P("phrnn_decd")
    Wxff, bxff, Whff, bhff = P("frnn_f")
    Wxfc, bxfc, Whfc, bhfc = P("frnn_c")
    Wm, bm = [np.asarray(x, np.float32) for x in p["bn_mean"]]
    Wlv, blv = [np.asarray(x, np.float32) for x in p["bn_logvar"]]

    xf = np.asarray(inputs["frnn_seq"], np.float32)
    xp = np.asarray(inputs["phrnn_seq"], np.float32)
    xs = np.asarray(inputs["sylrnn_seq"], np.float32)
    eps = np.asarray(inputs["eps"], np.float32)
    cf = np.asarray(inputs["frnn_clock"])
    cp = np.asarray(inputs["phrnn_clock"])
    sf = np.asarray(inputs["sample_freq"])
    dc = np.asarray(inputs["dec_clock"])
    dcc = np.asarray(inputs["dec_clock_c"])

    ts = np.arange(T)
    mf = ((ts % cf) == 0).astype(np.int8)
    mp = ((ts % cp) == 0).astype(np.int8)
    idx = np.nonzero(sf == 1)[0].astype(np.int64)
    K = len(idx)
    i_arr = np.arange(K)
    mcd = ((i_arr % dc[:K]) == 0).astype(np.float32)
    mcc = ((i_arr % dcc[:K]) == 0).astype(np.int8)
    cond = np.zeros(K, np.float32)
    cond[1:] = (sf[:K - 1] == 1).astype(np.float32)

    kb = [round(c * K / NCORES) for c in range(NCORES + 1)]

    # ---- weight blocks (shared) ----
    # W1full: out rows 0:64 = A-state pre-act recurrent part,
    #         rows 64:128 = B pre-act (x-part from A-state + recurrent part)
    wW1 = np.zeros((128, 128), np.float32)
    wW1[0:32, 0:32] = Whf0
    wW1[32:64, 32:64] = Whp0
    wW1[0:32, 64:96] = Wxf1
    wW1[32:64, 96:128] = Wxp1
    wW1[64:96, 64:96] = Whf1
    wW1[96:128, 96:128] = Whp1
    wWX1 = np.zeros((21, 128), np.float32)
    wWX1[0:13, 0:32] = Wxf0
    wWX1[13:20, 32:64] = Wxp0
    wWX1[20, 0:32] = bxf0 + bhf0
    wWX1[20, 32:64] = bxp0 + bhp0
    wWX1[20, 64:96] = bxf1 + bhf1
    wWX1[20, 96:128] = bxp1 + bhp1
    wWX = np.zeros((128, 128), np.float32)   # replicated per 64-row block
    for b in range(2):
        wWX[64 * b:64 * b + 21] = wWX1
    wSel = np.zeros((128, 32), np.float32)   # hf + hp fold for the record
    for i in range(H):
        wSel[64 + i, i] = 1.0
        wSel[96 + i, i] = 1.0

    Wu = np.concatenate([Wxff, Wxfc], axis=1)          # (32,13)
    bu = np.concatenate([bxff + bhff, bxfc + bhfc])    # (13,)
    DD = np.zeros((13, 13), np.float32)
    DD[0, 0] = Whff[0, 0]
    DD[1:, 1:] = Whfc

    weights = {
        "wW1": wW1, "wWX": wWX, "wSel": wSel,
        "wC1": _blkdiag4(Wsh), "wC2": _blkdiag4(Wxs),
        "wMm": _blkdiag4(Wm), "wMlv": _blkdiag4(Wlv),
        "wZ1": _blkdiag4(Wxs), "wZ2": _blkdiag4(Wxd),
        "wU": _blkdiag4(Wu), "wD": _blkdiag4(DD),
        "wEye": np.eye(52, dtype=np.float32),
    }
    bias = np.zeros((128, 8), np.float32)
    bias[:, 0] = _tile4(bxs + bhs)        # C chain / hs_new
    bias[:, 1] = _tile4(bm)               # mean
    bias[:, 2] = _tile4(0.5 * blv)        # 0.5*logvar bias for exp
    bias[:, 3] = _tile4(bxd + bhd)        # hp_new
    bias[0:52, 4] = _tile4(bu)            # UV bias
    weights["bias"] = bias

    # ---- per-core geometry ----
    geo = []
    for c in range(NCORES):
        k0, k1 = kb[c], kb[c + 1]
        assert k1 - k0 <= 4 * QREAL
        qs = [k0 + q * QREAL for q in range(4)]
        kwin0 = max(0, k0 - EXT - 32)
        start = 0 if c == 0 else int(idx[kwin0])
        assert int(idx[k1 - 1]) - start < ECOV, \
            f"core {c}: encoder window too small"
        geo.append({"k0": k0, "k1": k1, "qs": qs, "start": start})

    # uniform per-quarter record-window lane offsets (lane-aligned, shared
    # across cores so the program is SPMD-uniform)
    lq = []
    NLQ = 0
    for q in range(4):
        los, his = [], []
        for c in range(NCORES):
            g = geo[c]
            kq0 = max(0, g["qs"][q] - EXT - 32)
            kqL = min(g["k1"], g["qs"][q] + QREAL) - 1
            los.append(int(idx[kq0]) - g["start"])
            his.append(int(idx[kqL]) - g["start"])
        L_q = max(0, min(los) // LE)
        lq.append(L_q)
        NLQ = max(NLQ, -(-(max(his) + 1 - LE * L_q) // LE))
    NLQ += 1  # margin lane
    assert max(lq) + NLQ <= NRE, f"record window overflow {lq} {NLQ}"
    EQ = NLQ * LE

    # ---- per-core data ----
    cores = []
    for c in range(NCORES):
        g = geo[c]
        k0, k1, qs, start = g["k0"], g["k1"], g["qs"], g["start"]

        tloc = np.arange(EBUFX) - EPRE + start
        ok = (tloc >= 0) & (tloc < T)
        tc_ = np.clip(tloc, 0, T - 1)
        xfp_flat = np.zeros((21, EBUFX), np.float32)
        xfp_flat[0:13] = np.where(ok, xf[tc_].T, 0.0)
        xfp_flat[13:20] = np.where(ok, xp[tc_].T, 0.0)
        xfp_flat[20] = 1.0
        # 2-block layout: flat col j -> block j%2, col j//2
        u_xfp = np.zeros((128, XBUF2), np.float32)
        for b in range(2):
            cols = np.arange(b, EBUFX, 2)
            u_xfp[64 * b:64 * b + 21, 0:len(cols)] = xfp_flat[:, cols]
        msk_flat = np.zeros((64, EBUFM), np.int8)
        okm = ok[:EBUFM]
        tcm = tc_[:EBUFM]
        msk_flat[0:32] = np.where(okm, mf[tcm], 0)
        msk_flat[32:64] = np.where(okm, mp[tcm], 0)
        # 2-block layout (even/odd steps), A rows 0:64 and B rows 64:128
        u_mskE = np.zeros((128, MBUF2), np.int8)
        u_mskO = np.zeros((128, MBUF2), np.int8)
        u_mskE[0:64] = msk_flat[:, 0::2]
        u_mskE[64:128] = u_mskE[0:64]
        u_mskO[0:64] = msk_flat[:, 1::2]
        u_mskO[64:128] = u_mskO[0:64]

        u_gidx = np.zeros((128, QWB // 16), np.uint16)
        u_xsK = np.zeros((128, QWB), np.float32)
        u_eps = np.zeros((128, QW), np.float32)
        u_mcd = np.zeros((128, QW), np.float32)
        u_cnd = np.zeros((52, QW), np.float32)
        u_mcc = np.zeros((52, UVDW), np.int8)
        for q in range(4):
            # gather/C-chain col j (0..QWB) <-> k = qs[q] - EXT - 32 + j
            kk = qs[q] - EXT - 32 + np.arange(QWB)
            live = (kk >= 0) & (kk < k1)
            kkc = np.clip(kk, 0, K - 1)
            tk = np.where(live, idx[kkc] - start - LE * lq[q], 0).astype(np.int64)
            assert tk.min() >= 0 and tk.max() < EQ, f"core {c} q{q} idx range"
            w16 = tk.reshape(QWB // 16, 16).T.astype(np.uint16)
            u_gidx[32 * q:32 * q + 16] = w16
            u_gidx[32 * q + 16:32 * q + 32] = w16
            u_xsK[32 * q:32 * q + 32] = np.where(live, xs[idx[kkc]].T, 0.0)
            # ys/Z col j2 (0..QW) <-> k = qs[q] - EXT + j2
            kk2 = kk[32:]
            live2 = live[32:]
            kkc2 = kkc[32:]
            u_eps[32 * q:32 * q + 32] = np.where(live2, eps[kkc2].T, 0.0)
            u_mcd[32 * q:32 * q + 32] = np.where(live2, mcd[kkc2], 0.0)
            u_cnd[13 * q:13 * q + 13] = np.where(live2, cond[kkc2], 0.0)
            # UVD col j' <-> k = qs[q] + j' - 32
            kk3 = qs[q] + np.arange(UVDW) - 32
            live3 = (kk3 >= 0) & (kk3 < k1) & (np.arange(UVDW) >= DPAD)
            kkc3 = np.clip(kk3, 0, K - 1)
            u_mcc[13 * q, :] = np.where(live3, 1, 0)
            u_mcc[13 * q + 1:13 * q + 13, :] = np.where(live3, mcc[kkc3], 0)

        cores.append({
            "u_xfp": u_xfp, "u_mskE": u_mskE, "u_mskO": u_mskO,
            "u_gidx": u_gidx,
            "u_xsK": u_xsK, "u_eps": u_eps, "u_mcd": u_mcd,
            "u_cnd": u_cnd, "u_mcc": u_mcc,
        })

    return {"K": K, "kb": kb, "bm": bm, "blv": blv, "weights": weights,
            "geo": geo, "cores": cores, "lq": lq, "NLQ": NLQ, "EQ": EQ}


# ---------------- bass program ----------------

def build_program(lq, NLQ, EQ):
    nc = bacc.Bacc()

    din = {}
    for name, shape, dt in [
        ("u_xfp", (128, XBUF2), DT), ("u_mskE", (128, MBUF2), I8),
        ("u_mskO", (128, MBUF2), I8),
        ("u_gidx", (128, QWB // 16), U16),
        ("u_xsK", (128, QWB), DT), ("u_eps", (128, QW), DT),
        ("u_mcd", (128, QW), DT), ("u_cnd", (52, QW), DT),
        ("u_mcc", (52, UVDW), I8),
        ("wW1", (128, 128), DT), ("wWX", (128, 128), DT),
        ("wSel", (128, 32), DT),
        ("wC1", (128, 128), DT), ("wC2", (128, 128), DT),
        ("wMm", (128, 128), DT), ("wMlv", (128, 128), DT),
        ("wZ1", (128, 128), DT), ("wZ2", (128, 128), DT),
        ("wU", (128, 52), DT), ("wD", (52, 52), DT),
        ("wEye", (52, 52), DT), ("bias", (128, 8), DT),
    ]:
        din[name] = nc.declare_dram_parameter(name, list(shape), dt,
                                              isOutput=False)
    dout = {}
    outs = [("o_SD", (52, QLANES), DT), ("o_mean", (32, QW), DT),
            ("o_lv", (32, QW), DT)]
    if DEBUG_OUTS:
        outs += [("o_hq", (128, EQ), DT), ("o_ys", (128, QW), DT),
                 ("o_uvd", (52, UVDW), DT), ("o_resk", (128, QWB), DT)]
    for name, shape, dt in outs:
        dout[name] = nc.declare_dram_parameter(name, list(shape), dt,
                                               isOutput=True)

    with tile.TileContext(nc) as tc, ExitStack() as ctx:
        pc = ctx.enter_context(tc.tile_pool(name="const", bufs=1))
        po = ctx.enter_context(tc.tile_pool(name="outer", bufs=1))

        w = {}
        for name in ["wW1", "wWX", "wSel", "wC1", "wC2", "wMm", "wMlv",
                     "wZ1", "wZ2", "wU", "wD", "wEye", "bias", "u_gidx",
                     "u_xsK", "u_eps", "u_mcd", "u_cnd", "u_mcc"]:
            t = pc.tile(list(din[name].shape), din[name].dtype, tag=name)
            nc.gpsimd.dma_start(t[:], din[name][:])
            w[name] = t
        bias = w["bias"]

        # engine warm-ups: let each engine observe the DMA sems on a cheap op
        wrm = pc.tile([128, 8], DT)
        nc.vector.tensor_copy(wrm[0:52, 0:1], w["u_cnd"][:, 0:1])
        nc.vector.tensor_copy(wrm[:, 1:2], w["u_mcd"][:, 0:1])
        nc.vector.tensor_copy(wrm[:, 2:3], w["u_eps"][:, 0:1])
        nc.vector.tensor_copy(wrm[:, 3:4], w["u_xsK"][:, 0:1])
        wrm8 = pc.tile([128, 2], I8)
        nc.vector.tensor_copy(wrm8[0:52, 0:1], w["u_mcc"][:, 0:1])
        wrm16 = pc.tile([128, 1], U16)
        nc.gpsimd.tensor_copy(wrm16[:], w["u_gidx"][:, 0:1])
        # dummy gather: forces the gpsimd ap_gather library load early so it
        # overlaps phase E instead of blocking phase G
        zidx = pc.tile([128, 16], U16)
        nc.vector.memset(zidx[:], 0)
        gjunk = pc.tile([128, 256], DT)
        nc.vector.memset(gjunk[:], 0.0)
        nc.gpsimd.indirect_copy(gjunk[:, 0:16], gjunk[:], zidx[:, 0:1], True)

        # persistent k-space tiles
        ysK = po.tile([128, QW], DT)
        RESK = po.tile([128, QWB], DT)
        UV52 = po.tile([52, QW], DT)
        UVD = po.tile([52, UVDW], DT)

        # ---------------- phase E ----------------
        S128 = po.tile([128, ELANES], DT)   # rows 0:64 A, rows 64:128 B
        SHT = po.tile([128, ELANES], DT)
        nc.vector.memset(S128[:], 0.0)

        with tc.tile_pool(name="hq", bufs=1) as p_hq:
            HQR = p_hq.tile([128, EQ], DT)   # quarter-windowed hf+hp history
            with tc.tile_pool(name="ein", bufs=1) as p_ein, \
                 tc.tile_pool(name="est", bufs=3) as p_est, \
                 tc.tile_pool(name="e_ps", bufs=3, space="PSUM") as p_epp, \
                 tc.tile_pool(name="e_psr", bufs=3, space="PSUM") as p_epr:
                xfp = p_ein.tile([128, XBUF2], DT)
                mskE = p_ein.tile([128, MBUF2], I8)
                mskO = p_ein.tile([128, MBUF2], I8)
                nc.gpsimd.dma_start(xfp[:], din["u_xfp"][:])
                nc.gpsimd.dma_start(mskE[:], din["u_mskE"][:])
                nc.gpsimd.dma_start(mskO[:], din["u_mskO"][:])
                nc.vector.tensor_copy(wrm8[:, 1:2], mskE[:, 0:1])
                nc.vector.tensor_copy(wrm8[0:64, 0:1], mskO[0:64, 0:1])

                def mska(step):
                    t = mskE if step % 2 == 0 else mskO
                    return _cols(t[0:64, :], step // 2, LK, ELANES)

                def mskb(step):
                    t = mskE if step % 2 == 0 else mskO
                    return _cols(t[64:128, :], step // 2, LK, ELANES)

                for sweep in range(E_SWEEPS):
                    if sweep:
                        nc.vector.tensor_copy(SHT[:], S128[:])
                        nc.vector.tensor_copy(S128[:, 1:ELANES],
                                              SHT[:, 0:ELANES - 1])
                        nc.vector.memset(S128[:, 0:1], 0.0)
                    for tk in range(LE + 1):
                        a_on = tk < LE
                        b_on = tk >= 1
                        P = p_epp.tile([128, ELANES], DT)
                        b2 = 64 * (tk % 2)
                        nc.tensor.matmul(P[:], w["wWX"][b2:b2 + 21, :],
                                         _cols(xfp[b2:b2 + 21, :], tk // 2,
                                               32, ELANES),
                                         start=True, stop=False)
                        nc.tensor.matmul(P[:], w["wW1"][:], S128[:],
                                         start=False, stop=True)
                        T1 = p_est.tile([128, ELANES], DT)
                        nc.scalar.activation(T1[:], P[:], AF.Tanh)
                        if a_on:
                            nc.vector.copy_predicated(
                                S128[0:64, :], mska(tk), T1[0:64, :])
                        if b_on:
                            sB = tk - 1
                            nc.vector.copy_predicated(
                                S128[64:128, :], mskb(sB), T1[64:128, :])
                            nc.scalar.activation(S128[64:128, :],
                                                 S128[64:128, :], AF.Gelu)
                            if sweep == E_SWEEPS - 1:
                                PR = p_epr.tile([32, ELANES], DT)
                                nc.tensor.matmul(PR[:], w["wSel"][:],
                                                 S128[:], start=True,
                                                 stop=True)
                                for q in range(4):
                                    nc.vector.tensor_copy(
                                        _cols(HQR[32 * q:32 * q + 32, :],
                                              sB, LE, NLQ),
                                        PR[:, lq[q] + 1:lq[q] + NLQ + 1])
            if DEBUG_OUTS:
                nc.sync.dma_start(dout["o_hq"][:], HQR[:])

            # ---------------- phase G ----------------
            with tc.tile_pool(name="gat", bufs=1) as p_g:
                HFPK = p_g.tile([128, QWB], DT)
                nc.gpsimd.tensor_copy(wrm[:, 4:5], HQR[:, 0:1])
                NIC = 352  # indirect_copy output-width ISA cap is ~448
                for o in range(0, QWB, NIC):
                    nc.gpsimd.indirect_copy(
                        HFPK[:, o:o + NIC], HQR[:],
                        w["u_gidx"][:, o // 16:(o + NIC) // 16], True)
                nc.vector.tensor_add(RESK[:], HFPK[:], w["u_xsK"][:])
        if DEBUG_OUTS:
            nc.sync.dma_start(dout["o_resk"][:], RESK[:])

        # ---------------- phase C ----------------
        S_C = po.tile([128, QLANES], DT)
        nc.vector.memset(S_C[:], 0.0)
        SHC = po.tile([128, QLANES], DT)
        with tc.tile_pool(name="cst", bufs=3) as p_cst, \
             tc.tile_pool(name="c_ps", bufs=3, space="PSUM") as p_cpp:
            for sweep in range(C_SWEEPS):
                if sweep:
                    nc.vector.tensor_copy(SHC[:], S_C[:])
                    nc.vector.tensor_copy(S_C[:, 1:QLANES],
                                          SHC[:, 0:QLANES - 1])
                    nc.vector.memset(S_C[:, 0:1], 0.0)
                for s in range(LK):
                    P = p_cpp.tile([128, QLANES], DT)
                    nc.tensor.matmul(P[:], w["wC2"][:],
                                     _cols(RESK[:], s, LK, QLANES),
                                     start=True, stop=False)
                    nc.tensor.matmul(P[:], w["wC1"][:], S_C[:],
                                     start=False, stop=True)
                    TC = p_cst.tile([128, QLANES], DT)
                    nc.scalar.activation(TC[:], P[:], AF.Tanh,
                                         bias=bias[:, 0:1])
                    nc.scalar.activation(S_C[:], TC[:], AF.Gelu)
                    if sweep == C_SWEEPS - 1:
                        nc.vector.tensor_copy(_cols(ysK[:], s, LK, NRQ),
                                              S_C[:, 1:QLANES])
        if DEBUG_OUTS:
            nc.sync.dma_start(dout["o_ys"][:], ysK[:])

        # ---------------- phase Z ----------------
        SL = [(i, min(512, QW - i)) for i in range(0, QW, 512)]
        with tc.tile_pool(name="zbuf", bufs=1) as p_z, \
             tc.tile_pool(name="z_ps", bufs=4, space="PSUM") as p_zpp:
            MEAN = p_z.tile([128, QW], DT)
            LV = p_z.tile([128, QW], DT)
            ET = p_z.tile([128, QW], DT)
            PZ = p_z.tile([128, QW], DT)
            ZZ = p_z.tile([128, QW], DT)
            TS = p_z.tile([128, QW], DT)
            GS = p_z.tile([128, QW], DT)
            HSN = p_z.tile([128, QW], DT)
            HPN = p_z.tile([128, QW], DT)

            for dst, wm in ((MEAN, "wMm"), (LV, "wMlv")):
                for o, n in SL:
                    PM = p_zpp.tile([128, 512], DT)
                    nc.tensor.matmul(PM[:, 0:n], w[wm][:], ysK[:, o:o + n],
                                     start=True, stop=True)
                    nc.vector.tensor_copy(dst[:, o:o + n], PM[:, 0:n])
            nc.sync.dma_start(dout["o_mean"][:], MEAN[96:128, :])
            nc.sync.dma_start(dout["o_lv"][:], LV[96:128, :])
            nc.scalar.activation(ET[:], LV[:], AF.Exp,
                                 bias=bias[:, 2:3], scale=0.5)
            nc.vector.tensor_mul(PZ[:], ET[:], w["u_eps"][:])
            nc.vector.scalar_tensor_tensor(
                ZZ[:], MEAN[:], bias[:, 1:2], PZ[:],
                op0=mybir.AluOpType.add, op1=mybir.AluOpType.add)
            for o, n in SL:
                PM = p_zpp.tile([128, 512], DT)
                nc.tensor.matmul(PM[:, 0:n], w["wZ1"][:], ZZ[:, o:o + n],
                                 start=True, stop=True)
                nc.scalar.activation(TS[:, o:o + n], PM[:, 0:n], AF.Tanh,
                                     bias=bias[:, 0:1])
            nc.scalar.activation(GS[:], TS[:], AF.Gelu)
            nc.vector.tensor_mul(HSN[:], GS[:], w["u_mcd"][:])
            for o, n in SL:
                PM = p_zpp.tile([128, 512], DT)
                nc.tensor.matmul(PM[:, 0:n], w["wZ2"][:], HSN[:, o:o + n],
                                 start=True, stop=True)
                nc.scalar.activation(TS[:, o:o + n], PM[:, 0:n], AF.Tanh,
                                     bias=bias[:, 3:4])
            nc.scalar.activation(GS[:], TS[:], AF.Gelu)
            nc.vector.tensor_mul(HPN[:], GS[:], w["u_mcd"][:])
            for o, n in SL:
                PU = p_zpp.tile([128, 512], DT)
                nc.tensor.matmul(PU[0:52, 0:n], w["wU"][:], HPN[:, o:o + n],
                                 start=True, stop=True)
                nc.scalar.activation(UV52[:, o:o + n], PU[0:52, 0:n],
                                     AF.Identity, bias=bias[0:52, 4:5])
            CM = p_z.tile([52, QW], DT)
            CUV = p_z.tile([52, QW], DT)
            nc.vector.tensor_scalar(CM[:], w["u_cnd"][:], -1.0, 1.0,
                                    op0=mybir.AluOpType.mult,
                                    op1=mybir.AluOpType.add)
            nc.vector.tensor_mul(CUV[:], UV52[:], w["u_cnd"][:])
            nc.vector.memset(UVD[:, 0:DPAD], 0.0)
            nc.vector.memset(UVD[:, DPAD + QW:], 0.0)
            nc.vector.tensor_tensor_scan(
                UVD[:, DPAD:DPAD + QW], CM[:], CUV[:], 0.0,
                op0=mybir.AluOpType.mult, op1=mybir.AluOpType.add)
        if DEBUG_OUTS:
            nc.sync.dma_start(dout["o_uvd"][:], UVD[:])

        # ---------------- phase D ----------------
        S_D = po.tile([52, QLANES], DT)
        nc.vector.memset(S_D[:], 0.0)
        SHD = po.tile([52, QLANES], DT)
        with tc.tile_pool(name="dst", bufs=3) as p_dst, \
             tc.tile_pool(name="d_ps", bufs=3, space="PSUM") as p_dpp:
            for sweep in range(D_SWEEPS):
                if sweep:
                    nc.vector.tensor_copy(SHD[:], S_D[:])
                    nc.vector.tensor_copy(S_D[:, 1:QLANES],
                                          SHD[:, 0:QLANES - 1])
                    nc.vector.memset(S_D[:, 0:1], 0.0)
                for s in range(LK):
                    P = p_dpp.tile([52, QLANES], DT)
                    nc.tensor.matmul(P[:], w["wEye"][:],
                                     _cols(UVD[:], s, LK, QLANES),
                                     start=True, stop=False)
                    nc.tensor.matmul(P[:], w["wD"][:], S_D[:],
                                     start=False, stop=True)
                    TD = p_dst.tile([52, QLANES], DT)
                    nc.scalar.activation(TD[:], P[:], AF.Tanh)
                    nc.vector.copy_predicated(
                        S_D[:], _cols(w["u_mcc"][:], s, LK, QLANES), TD[:])
        nc.sync.dma_start(dout["o_SD"][:], S_D[:])

    nc.finalize()
    return nc


# ---------------- entry point ----------------

def kernel(**inputs):
    host = _prep(inputs)
    nc = build_program(host["lq"], host["NLQ"], host["EQ"])

    in_maps = []
    for c in range(NCORES):
        m = {k: np.ascontiguousarray(v) for k, v in host["weights"].items()}
        for name, v in host["cores"][c].items():
            m[name] = np.ascontiguousarray(v)
        in_maps.append(m)

    from concourse.bass_utils import run_bass_kernel_spmd
    res = run_bass_kernel_spmd(nc, in_maps, list(range(NCORES)),
                               trace=TRACE)
    LAST["exec_time_ns"] = res.exec_time_ns
    LAST["results"] = res.results
    LAST["insts"] = res.instructions_and_trace

    K = host["K"]
    g7 = host["geo"][NCORES - 1]
    qs3 = g7["qs"][3]
    r = res.results[NCORES - 1]
    lane = (K - 1 - qs3) // LK + 1
    SD = np.asarray(r["o_SD"])
    hff = np.array([SD[39, lane]], np.float32)
    hfc = SD[40:52, lane].astype(np.float32)
    j = K - 1 - qs3 + EXT
    mean = np.asarray(r["o_mean"])[:, j] + host["bm"]
    lv = np.asarray(r["o_lv"])[:, j] + host["blv"]
    kl = np.float32(-0.5 * np.sum(1.0 + lv - mean * mean - np.exp(lv)))
    return hff, hfc, kl


# revision 12
# speedup vs baseline: 1.5242x; 1.0466x over previous
"""Trainium2 Bass kernel for nn_CHIVE_53111565583018 (clockwork-RNN CHIVE).

The model is a strictly sequential scan (T=131072 encoder steps, K~65536
decoder steps) with tiny (<=32-dim) state, but every chain is strongly
contracting (tanh/gelu + clockwork holds forget initial conditions fast).
We run *chunked* scans: each core processes its time/k range as ~256 chunks
held in the matmul free dimension, advancing all chunks in lockstep, with
2-3 "sweeps" where sweep s re-seeds chunk i with chunk i-1's final state
from sweep s-1.  Boundary error after S sweeps ~ contraction(L)^S, validated
offline (final output rel-err <= ~1e-3).

Per-core phases (8 cores SPMD; core c owns 1/8 of the decoder k-range and an
encoder time window covering it):
  E: encoder frnn/phrnn chains, 64-step chunks x 265 lanes, 2 sweeps;
     records hf+hp per step (via a selection matmul).
  G: repack history to quarter-packed layout (DMA) + indirect_copy gather
     of hf+hp at syllable positions (k-space).
  C: sylrnn hs chain, dense in k-space, 32-step chunks x 4 quarter-chains
     packed on partitions, 2 sweeps; records ys.
  Z: decoder parallel math: mean/logvar/z/hs_new/hp_new/UV projections and
     the cond fill-forward (tensor_tensor_scan).
  D: hff/hfc chain, 32-step chunks, 3 sweeps.
Final outputs (hff[1], hfc[12], kl) assembled on host from core 7's exports.
"""
import os
import sys
import numpy as np
from contextlib import ExitStack

sys.path.insert(0, "/opt/trn_rl_repo")

import concourse.bass as bass
import concourse.bacc as bacc
import concourse.tile as tile
from concourse import mybir

# ---------------- constants (problem-specific, hardcoded) ----------------
T = 131072
H = 32
NCORES = 8

LE = 66            # encoder chunk length (even; lanes*1 matmul = 256 cols)
NRE = 255          # encoder real lanes/core
ELANES = NRE + 1   # + prefix lane = 256
ECOV = NRE * LE    # 16830 encoder steps covered per core
EPRE = LE          # leading prefix columns in encoder buffers
EBUFM = EPRE + ECOV           # mask buffer cols
EBUFX = EPRE + ECOV + 2 * LE  # xfp buffer cols (tail pad for trailing ticks)

LK = 32            # k-space chunk length (C and D chains)
QREAL = 2056       # real k slots per quarter
EXT = 24           # leading real-k extension per quarter
QW = EXT + QREAL   # 2080 recorded k slots per quarter
QWB = 32 + QW      # 2112: + prefix-lane slots (C chain / gather)
NRQ = QW // LK     # 65 real lanes per quarter chain
QLANES = NRQ + 1   # 66
DPAD = LK - EXT    # 8 zero cols ahead of UVD
UVDW = DPAD + QW + EXT  # 2112
XBUF2 = EBUFX // 2 + 32   # 2-block xfp layout cols
MBUF2 = EBUFM // 2        # 2-block mask layout cols

E_SWEEPS = 1
C_SWEEPS = 1
D_SWEEPS = 3

DT = mybir.dt.float32
I8 = mybir.dt.int8
U16 = mybir.dt.uint16
AF = mybir.ActivationFunctionType

TRACE = bool(int(os.environ.get("KERNEL_TRACE", "0")))
DEBUG_OUTS = bool(int(os.environ.get("KERNEL_DEBUG_OUTS", "0")))
LAST = {}  # exec info for the test harness


def _cols(ap2d, start, stride, count):
    return ap2d[:, start: start + (count - 1) * stride + 1: stride]


def _blkdiag4(w):
    n, m = w.shape
    out = np.zeros((4 * n, 4 * m), np.float32)
    for q in range(4):
        out[q * n:(q + 1) * n, q * m:(q + 1) * m] = w
    return out


def _tile4(v):
    return np.tile(np.asarray(v, np.float32), 4)


# ---------------- host-side preprocessing ----------------

def _prep(inputs):
    p = inputs["params"]

    def P(name):
        d = p[name]
        return (np.asarray(d["Wx"], np.float32), np.asarray(d["bx"], np.float32),
                np.asarray(d["Wh"], np.float32), np.asarray(d["bh"], np.float32))

    Wxf0, bxf0, Whf0, bhf0 = P("frnn0")
    Wxf1, bxf1, Whf1, bhf1 = P("frnn1")
    Wxp0, bxp0, Whp0, bhp0 = P("phrnn0")
    Wxp1, bxp1, Whp1, bhp1 = P("phrnn1")
    Wxs, bxs, Wsh, bhs = P("sylrnn")
    Wxd, bxd, _Whd, bhd = P("phrnn_decd")
    Wxff, bxff, Whff, bhff = P("frnn_f")
    Wxfc, bxfc, Whfc, bhfc = P("frnn_c")
    Wm, bm = [np.asarray(x, np.float32) for x in p["bn_mean"]]
    Wlv, blv = [np.asarray(x, np.float32) for x in p["bn_logvar"]]

    xf = np.asarray(inputs["frnn_seq"], np.float32)
    xp = np.asarray(inputs["phrnn_seq"], np.float32)
    xs = np.asarray(inputs["sylrnn_seq"], np.float32)
    eps = np.asarray(inputs["eps"], np.float32)
    cf = np.asarray(inputs["frnn_clock"])
    cp = np.asarray(inputs["phrnn_clock"])
    sf = np.asarray(inputs["sample_freq"])
    dc = np.asarray(inputs["dec_clock"])
    dcc = np.asarray(inputs["dec_clock_c"])

    ts = np.arange(T)
    mf = ((ts % cf) == 0).astype(np.int8)
    mp = ((ts % cp) == 0).astype(np.int8)
    idx = np.nonzero(sf == 1)[0].astype(np.int64)
    K = len(idx)
    i_arr = np.arange(K)
    mcd = ((i_arr % dc[:K]) == 0).astype(np.float32)
    mcc = ((i_arr % dcc[:K]) == 0).astype(np.int8)
    cond = np.zeros(K, np.float32)
    cond[1:] = (sf[:K - 1] == 1).astype(np.float32)

    kb = [round(c * K / NCORES) for c in range(NCORES + 1)]

    # ---- weight blocks (shared) ----
    # W1full: out rows 0:64 = A-state pre-act recurrent part,
    #         rows 64:128 = B pre-act (x-part from A-state + recurrent part)
    wW1 = np.zeros((128, 128), np.float32)
    wW1[0:32, 0:32] = Whf0
    wW1[32:64, 32:64] = Whp0
    wW1[0:32, 64:96] = Wxf1
    wW1[32:64, 96:128] = Wxp1
    wW1[64:96, 64:96] = Whf1
    wW1[96:128, 96:128] = Whp1
    wWX1 = np.zeros((21, 128), np.float32)
    wWX1[0:13, 0:32] = Wxf0
    wWX1[13:20, 32:64] = Wxp0
    wWX1[20, 0:32] = bxf0 + bhf0
    wWX1[20, 32:64] = bxp0 + bhp0
    wWX1[20, 64:96] = bxf1 + bhf1
    wWX1[20, 96:128] = bxp1 + bhp1
    wWX = np.zeros((128, 128), np.float32)   # replicated per 64-row block
    for b in range(2):
        wWX[64 * b:64 * b + 21] = wWX1
    wSel = np.zeros((128, 32), np.float32)   # hf + hp fold for the record
    for i in range(H):
        wSel[64 + i, i] = 1.0
        wSel[96 + i, i] = 1.0

    Wu = np.concatenate([Wxff, Wxfc], axis=1)          # (32,13)
    bu = np.concatenate([bxff + bhff, bxfc + bhfc])    # (13,)
    DD = np.zeros((13, 13), np.float32)
    DD[0, 0] = Whff[0, 0]
    DD[1:, 1:] = Whfc

    weights = {
        "wW1": wW1, "wWX": wWX, "wSel": wSel,
        "wC1": _blkdiag4(Wsh), "wC2": _blkdiag4(Wxs),
        "wMm": _blkdiag4(Wm), "wMlv": _blkdiag4(Wlv),
        "wZ1": _blkdiag4(Wxs), "wZ2": _blkdiag4(Wxd),
        "wU": _blkdiag4(Wu), "wD": _blkdiag4(DD),
        "wEye": np.eye(52, dtype=np.float32),
    }
    bias = np.zeros((128, 8), np.float32)
    bias[:, 0] = _tile4(bxs + bhs)        # C chain / hs_new
    bias[:, 1] = _tile4(bm)               # mean
    bias[:, 2] = _tile4(0.5 * blv)        # 0.5*logvar bias for exp
    bias[:, 3] = _tile4(bxd + bhd)        # hp_new
    bias[0:52, 4] = _tile4(bu)            # UV bias
    weights["bias"] = bias

    # ---- per-core geometry ----
    geo = []
    for c in range(NCORES):
        k0, k1 = kb[c], kb[c + 1]
        assert k1 - k0 <= 4 * QREAL
        qs = [k0 + q * QREAL for q in range(4)]
        kwin0 = max(0, k0 - EXT - 32)
        start = 0 if c == 0 else int(idx[kwin0])
        assert int(idx[k1 - 1]) - start < ECOV, \
            f"core {c}: encoder window too small"
        geo.append({"k0": k0, "k1": k1, "qs": qs, "start": start})

    # uniform per-quarter record-window lane offsets (lane-aligned, shared
    # across cores so the program is SPMD-uniform)
    lq = []
    NLQ = 0
    for q in range(4):
        los, his = [], []
        for c in range(NCORES):
            g = geo[c]
            kq0 = max(0, g["qs"][q] - EXT - 32)
            kqL = min(g["k1"], g["qs"][q] + QREAL) - 1
            los.append(int(idx[kq0]) - g["start"])
            his.append(int(idx[kqL]) - g["start"])
        L_q = max(0, min(los) // LE)
        lq.append(L_q)
        NLQ = max(NLQ, -(-(max(his) + 1 - LE * L_q) // LE))
    NLQ += 1  # margin lane
    assert max(lq) + NLQ <= NRE, f"record window overflow {lq} {NLQ}"
    EQ = NLQ * LE

    # ---- per-core data ----
    cores = []
    for c in range(NCORES):
        g = geo[c]
        k0, k1, qs, start = g["k0"], g["k1"], g["qs"], g["start"]

        tloc = np.arange(EBUFX) - EPRE + start
        ok = (tloc >= 0) & (tloc < T)
        tc_ = np.clip(tloc, 0, T - 1)
        xfp_flat = np.zeros((21, EBUFX), np.float32)
        xfp_flat[0:13] = np.where(ok, xf[tc_].T, 0.0)
        xfp_flat[13:20] = np.where(ok, xp[tc_].T, 0.0)
        xfp_flat[20] = 1.0
        # 2-block layout: flat col j -> block j%2, col j//2
        u_xfp = np.zeros((128, XBUF2), np.float32)
        for b in range(2):
            cols = np.arange(b, EBUFX, 2)
            u_xfp[64 * b:64 * b + 21, 0:len(cols)] = xfp_flat[:, cols]
        msk_flat = np.zeros((64, EBUFM), np.int8)
        okm = ok[:EBUFM]
        tcm = tc_[:EBUFM]
        msk_flat[0:32] = np.where(okm, mf[tcm], 0)
        msk_flat[32:64] = np.where(okm, mp[tcm], 0)
        # 2-block layout (even/odd steps), A rows 0:64 and B rows 64:128
        u_mskE = np.zeros((128, MBUF2), np.int8)
        u_mskO = np.zeros((128, MBUF2), np.int8)
        u_mskE[0:64] = msk_flat[:, 0::2]
        u_mskE[64:128] = u_mskE[0:64]
        u_mskO[0:64] = msk_flat[:, 1::2]
        u_mskO[64:128] = u_mskO[0:64]

        u_gidx = np.zeros((128, QWB // 16), np.uint16)
        u_xsK = np.zeros((128, QWB), np.float32)
        u_eps = np.zeros((128, QW), np.float32)
        u_mcd = np.zeros((128, QW), np.float32)
        u_cnd = np.zeros((52, QW), np.float32)
        u_mcc = np.zeros((52, UVDW), np.int8)
        for q in range(4):
            # gather/C-chain col j (0..QWB) <-> k = qs[q] - EXT - 32 + j
            kk = qs[q] - EXT - 32 + np.arange(QWB)
            live = (kk >= 0) & (kk < k1)
            kkc = np.clip(kk, 0, K - 1)
            tk = np.where(live, idx[kkc] - start - LE * lq[q], 0).astype(np.int64)
            assert tk.min() >= 0 and tk.max() < EQ, f"core {c} q{q} idx range"
            w16 = tk.reshape(QWB // 16, 16).T.astype(np.uint16)
            u_gidx[32 * q:32 * q + 16] = w16
            u_gidx[32 * q + 16:32 * q + 32] = w16
            u_xsK[32 * q:32 * q + 32] = np.where(live, xs[idx[kkc]].T, 0.0)
            # ys/Z col j2 (0..QW) <-> k = qs[q] - EXT + j2
            kk2 = kk[32:]
            live2 = live[32:]
            kkc2 = kkc[32:]
            u_eps[32 * q:32 * q + 32] = np.where(live2, eps[kkc2].T, 0.0)
            u_mcd[32 * q:32 * q + 32] = np.where(live2, mcd[kkc2], 0.0)
            u_cnd[13 * q:13 * q + 13] = np.where(live2, cond[kkc2], 0.0)
            # UVD col j' <-> k = qs[q] + j' - 32
            kk3 = qs[q] + np.arange(UVDW) - 32
            live3 = (kk3 >= 0) & (kk3 < k1) & (np.arange(UVDW) >= DPAD)
            kkc3 = np.clip(kk3, 0, K - 1)
            u_mcc[13 * q, :] = np.where(live3, 1, 0)
            u_mcc[13 * q + 1:13 * q + 13, :] = np.where(live3, mcc[kkc3], 0)

        cores.append({
            "u_xfp": u_xfp, "u_mskE": u_mskE, "u_mskO": u_mskO,
            "u_gidx": u_gidx,
            "u_xsK": u_xsK, "u_eps": u_eps, "u_mcd": u_mcd,
            "u_cnd": u_cnd, "u_mcc": u_mcc,
        })

    return {"K": K, "kb": kb, "bm": bm, "blv": blv, "weights": weights,
            "geo": geo, "cores": cores, "lq": lq, "NLQ": NLQ, "EQ": EQ}


# ---------------- bass program ----------------

def build_program(lq, NLQ, EQ):
    nc = bacc.Bacc()

    din = {}
    for name, shape, dt in [
        ("u_xfp", (128, XBUF2), DT), ("u_mskE", (128, MBUF2), I8),
        ("u_mskO", (128, MBUF2), I8),
        ("u_gidx", (128, QWB // 16), U16),
        ("u_xsK", (128, QWB), DT), ("u_eps", (128, QW), DT),
        ("u_mcd", (128, QW), DT), ("u_cnd", (52, QW), DT),
        ("u_mcc", (52, UVDW), I8),
        ("wW1", (128, 128), DT), ("wWX", (128, 128), DT),
        ("wSel", (128, 32), DT),
        ("wC1", (128, 128), DT), ("wC2", (128, 128), DT),
        ("wMm", (128, 128), DT), ("wMlv", (128, 128), DT),
        ("wZ1", (128, 128), DT), ("wZ2", (128, 128), DT),
        ("wU", (128, 52), DT), ("wD", (52, 52), DT),
        ("wEye", (52, 52), DT), ("bias", (128, 8), DT),
    ]:
        din[name] = nc.declare_dram_parameter(name, list(shape), dt,
                                              isOutput=False)
    dout = {}
    outs = [("o_SD", (52, QLANES), DT), ("o_mean", (32, QW), DT),
            ("o_lv", (32, QW), DT)]
    if DEBUG_OUTS:
        outs += [("o_hq", (128, EQ), DT), ("o_ys", (128, QW), DT),
                 ("o_uvd", (52, UVDW), DT), ("o_resk", (128, QWB), DT)]
    for name, shape, dt in outs:
        dout[name] = nc.declare_dram_parameter(name, list(shape), dt,
                                               isOutput=True)

    with tile.TileContext(nc) as tc, ExitStack() as ctx:
        pc = ctx.enter_context(tc.tile_pool(name="const", bufs=1))
        po = ctx.enter_context(tc.tile_pool(name="outer", bufs=1))

        w = {}
        for name in ["wW1", "wWX", "wSel", "wC1", "wC2", "wMm", "wMlv",
                     "wZ1", "wZ2", "wU", "wD", "wEye", "bias", "u_gidx",
                     "u_xsK", "u_eps", "u_mcd", "u_cnd", "u_mcc"]:
            t = pc.tile(list(din[name].shape), din[name].dtype, tag=name)
            nc.gpsimd.dma_start(t[:], din[name][:])
            w[name] = t
        bias = w["bias"]

        # engine warm-ups: let each engine observe the DMA sems on a cheap op
        wrm = pc.tile([128, 8], DT)
        nc.vector.tensor_copy(wrm[0:52, 0:1], w["u_cnd"][:, 0:1])
        nc.vector.tensor_copy(wrm[:, 1:2], w["u_mcd"][:, 0:1])
        nc.vector.tensor_copy(wrm[:, 2:3], w["u_eps"][:, 0:1])
        nc.vector.tensor_copy(wrm[:, 3:4], w["u_xsK"][:, 0:1])
        wrm8 = pc.tile([128, 2], I8)
        nc.vector.tensor_copy(wrm8[0:52, 0:1], w["u_mcc"][:, 0:1])
        wrm16 = pc.tile([128, 1], U16)
        nc.vector.tensor_copy(wrm16[:], w["u_gidx"][:, 0:1])
        # dummy gather: forces the gpsimd ap_gather library load early so it
        # overlaps phase E instead of blocking phase G
        zidx = pc.tile([128, 16], U16)
        nc.vector.memset(zidx[:], 0)
        gjunk = pc.tile([128, 256], DT)
        nc.vector.memset(gjunk[:], 0.0)
        nc.gpsimd.indirect_copy(gjunk[:, 0:16], gjunk[:], zidx[:, 0:1], True)

        # persistent k-space tiles
        ysK = po.tile([128, QW], DT)
        RESK = po.tile([128, QWB], DT)
        UV52 = po.tile([52, QW], DT)
        UVD = po.tile([52, UVDW], DT)

        # ---------------- phase E ----------------
        S128 = po.tile([128, ELANES], DT)   # rows 0:64 A, rows 64:128 B
        SHT = po.tile([128, ELANES], DT)
        nc.vector.memset(S128[:], 0.0)

        with tc.tile_pool(name="hq", bufs=1) as p_hq:
            HQR = p_hq.tile([128, EQ], DT)   # quarter-windowed hf+hp history
            with tc.tile_pool(name="ein", bufs=1) as p_ein, \
                 tc.tile_pool(name="est", bufs=3) as p_est, \
                 tc.tile_pool(name="e_ps", bufs=3, space="PSUM") as p_epp, \
                 tc.tile_pool(name="e_psr", bufs=3, space="PSUM") as p_epr:
                xfp = p_ein.tile([128, XBUF2], DT)
                mskE = p_ein.tile([128, MBUF2], I8)
                mskO = p_ein.tile([128, MBUF2], I8)
                nc.gpsimd.dma_start(xfp[:], din["u_xfp"][:])
                nc.gpsimd.dma_start(mskE[:], din["u_mskE"][:])
                nc.gpsimd.dma_start(mskO[:], din["u_mskO"][:])
                nc.vector.tensor_copy(wrm8[:, 1:2], mskE[:, 0:1])
                nc.vector.tensor_copy(wrm8[0:64, 0:1], mskO[0:64, 0:1])

                def mska(step):
                    t = mskE if step % 2 == 0 else mskO
                    return _cols(t[0:64, :], step // 2, LE // 2, ELANES)

                def mskb(step):
                    t = mskE if step % 2 == 0 else mskO
                    return _cols(t[64:128, :], step // 2, LE // 2, ELANES)

                for sweep in range(E_SWEEPS):
                    if sweep:
                        nc.vector.tensor_copy(SHT[:], S128[:])
                        nc.vector.tensor_copy(S128[:, 1:ELANES],
                                              SHT[:, 0:ELANES - 1])
                        nc.vector.memset(S128[:, 0:1], 0.0)
                    for tk in range(LE + 2):
                        a_on = tk < LE
                        b_on = 1 <= tk <= LE
                        r_on = sweep == E_SWEEPS - 1 and 2 <= tk
                        if a_on or b_on:
                            P = p_epp.tile([128, ELANES], DT)
                            b2 = 64 * (tk % 2)
                            nc.tensor.matmul(P[:], w["wWX"][b2:b2 + 21, :],
                                             _cols(xfp[b2:b2 + 21, :],
                                                   tk // 2, LE // 2, ELANES),
                                             start=True, stop=False)
                            nc.tensor.matmul(P[:], w["wW1"][:], S128[:],
                                             start=False, stop=True)
                        if r_on:
                            # record hf+hp of step tk-2: S128 B-rows still
                            # hold that state until this tick's cpB
                            sR = tk - 2
                            PR = p_epr.tile([32, ELANES], DT)
                            nc.tensor.matmul(PR[:], w["wSel"][:], S128[:],
                                             start=True, stop=True)
                            for q in range(4):
                                nc.vector.tensor_copy(
                                    _cols(HQR[32 * q:32 * q + 32, :],
                                          sR, LE, NLQ),
                                    PR[:, lq[q] + 1:lq[q] + NLQ + 1])
                        if a_on or b_on:
                            T1 = p_est.tile([128, ELANES], DT)
                            nc.scalar.activation(T1[:], P[:], AF.Tanh)
                        if b_on:
                            sB = tk - 1
                            nc.vector.copy_predicated(
                                S128[64:128, :], mskb(sB), T1[64:128, :])
                        if a_on:
                            nc.vector.copy_predicated(
                                S128[0:64, :], mska(tk), T1[0:64, :])
                        if b_on:
                            nc.scalar.activation(S128[64:128, :],
                                                 S128[64:128, :], AF.Gelu)
            if DEBUG_OUTS:
                nc.sync.dma_start(dout["o_hq"][:], HQR[:])

            # ---------------- phase G ----------------
            with tc.tile_pool(name="gat", bufs=1) as p_g:
                HFPK = p_g.tile([128, QWB], DT)
                nc.vector.tensor_copy(wrm[:, 4:5], HQR[:, 0:1])
                NIC = 352  # indirect_copy output-width ISA cap is ~448
                for o in range(0, QWB, NIC):
                    nc.gpsimd.indirect_copy(
                        HFPK[:, o:o + NIC], HQR[:],
                        w["u_gidx"][:, o // 16:(o + NIC) // 16], True)
                nc.vector.tensor_add(RESK[:], HFPK[:], w["u_xsK"][:])
        if DEBUG_OUTS:
            nc.sync.dma_start(dout["o_resk"][:], RESK[:])

        # ---------------- phase C ----------------
        S_C = po.tile([128, QLANES], DT)
        nc.vector.memset(S_C[:], 0.0)
        SHC = po.tile([128, QLANES], DT)
        with tc.tile_pool(name="cst", bufs=3) as p_cst, \
             tc.tile_pool(name="c_ps", bufs=3, space="PSUM") as p_cpp:
            for sweep in range(C_SWEEPS):
                if sweep:
                    nc.vector.tensor_copy(SHC[:], S_C[:])
                    nc.vector.tensor_copy(S_C[:, 1:QLANES],
                                          SHC[:, 0:QLANES - 1])
                    nc.vector.memset(S_C[:, 0:1], 0.0)
                for s in range(LK):
                    P = p_cpp.tile([128, QLANES], DT)
                    nc.tensor.matmul(P[:], w["wC2"][:],
                                     _cols(RESK[:], s, LK, QLANES),
                                     start=True, stop=False)
                    nc.tensor.matmul(P[:], w["wC1"][:], S_C[:],
                                     start=False, stop=True)
                    TC = p_cst.tile([128, QLANES], DT)
                    nc.scalar.activation(TC[:], P[:], AF.Tanh,
                                         bias=bias[:, 0:1])
                    nc.scalar.activation(S_C[:], TC[:], AF.Gelu)
                    if sweep == C_SWEEPS - 1:
                        nc.vector.tensor_copy(_cols(ysK[:], s, LK, NRQ),
                                              S_C[:, 1:QLANES])
        if DEBUG_OUTS:
            nc.sync.dma_start(dout["o_ys"][:], ysK[:])

        # ---------------- phase Z ----------------
        SL = [(i, min(512, QW - i)) for i in range(0, QW, 512)]
        with tc.tile_pool(name="zbuf", bufs=1) as p_z, \
             tc.tile_pool(name="z_ps", bufs=4, space="PSUM") as p_zpp:
            MEAN = p_z.tile([128, QW], DT)
            LV = p_z.tile([128, QW], DT)
            ET = p_z.tile([128, QW], DT)
            PZ = p_z.tile([128, QW], DT)
            ZZ = p_z.tile([128, QW], DT)
            TS = p_z.tile([128, QW], DT)
            GS = p_z.tile([128, QW], DT)
            HSN = p_z.tile([128, QW], DT)
            HPN = p_z.tile([128, QW], DT)

            for dst, wm in ((MEAN, "wMm"), (LV, "wMlv")):
                for o, n in SL:
                    PM = p_zpp.tile([128, 512], DT)
                    nc.tensor.matmul(PM[:, 0:n], w[wm][:], ysK[:, o:o + n],
                                     start=True, stop=True)
                    nc.vector.tensor_copy(dst[:, o:o + n], PM[:, 0:n])
            nc.sync.dma_start(dout["o_mean"][:], MEAN[96:128, :])
            nc.sync.dma_start(dout["o_lv"][:], LV[96:128, :])
            nc.scalar.activation(ET[:], LV[:], AF.Exp,
                                 bias=bias[:, 2:3], scale=0.5)
            nc.vector.tensor_mul(PZ[:], ET[:], w["u_eps"][:])
            nc.vector.scalar_tensor_tensor(
                ZZ[:], MEAN[:], bias[:, 1:2], PZ[:],
                op0=mybir.AluOpType.add, op1=mybir.AluOpType.add)
            for o, n in SL:
                PM = p_zpp.tile([128, 512], DT)
                nc.tensor.matmul(PM[:, 0:n], w["wZ1"][:], ZZ[:, o:o + n],
                                 start=True, stop=True)
                nc.scalar.activation(TS[:, o:o + n], PM[:, 0:n], AF.Tanh,
                                     bias=bias[:, 0:1])
            nc.scalar.activation(GS[:], TS[:], AF.Gelu)
            nc.vector.tensor_mul(HSN[:], GS[:], w["u_mcd"][:])
            for o, n in SL:
                PM = p_zpp.tile([128, 512], DT)
                nc.tensor.matmul(PM[:, 0:n], w["wZ2"][:], HSN[:, o:o + n],
                                 start=True, stop=True)
                nc.scalar.activation(TS[:, o:o + n], PM[:, 0:n], AF.Tanh,
                                     bias=bias[:, 3:4])
            nc.scalar.activation(GS[:], TS[:], AF.Gelu)
            nc.vector.tensor_mul(HPN[:], GS[:], w["u_mcd"][:])
            for o, n in SL:
                PU = p_zpp.tile([128, 512], DT)
                nc.tensor.matmul(PU[0:52, 0:n], w["wU"][:], HPN[:, o:o + n],
                                 start=True, stop=True)
                nc.scalar.activation(UV52[:, o:o + n], PU[0:52, 0:n],
                                     AF.Identity, bias=bias[0:52, 4:5])
            CM = p_z.tile([52, QW], DT)
            CUV = p_z.tile([52, QW], DT)
            nc.vector.tensor_scalar(CM[:], w["u_cnd"][:], -1.0, 1.0,
                                    op0=mybir.AluOpType.mult,
                                    op1=mybir.AluOpType.add)
            nc.vector.tensor_mul(CUV[:], UV52[:], w["u_cnd"][:])
            nc.vector.memset(UVD[:, 0:DPAD], 0.0)
            nc.vector.memset(UVD[:, DPAD + QW:], 0.0)
            nc.vector.tensor_tensor_scan(
                UVD[:, DPAD:DPAD + QW], CM[:], CUV[:], 0.0,
                op0=mybir.AluOpType.mult, op1=mybir.AluOpType.add)
        if DEBUG_OUTS:
            nc.sync.dma_start(dout["o_uvd"][:], UVD[:])

        # ---------------- phase D ----------------
        S_D = po.tile([52, QLANES], DT)
        nc.vector.memset(S_D[:], 0.0)
        SHD = po.tile([52, QLANES], DT)
        with tc.tile_pool(name="dst", bufs=3) as p_dst, \
             tc.tile_pool(name="d_ps", bufs=3, space="PSUM") as p_dpp:
            for sweep in range(D_SWEEPS):
                if sweep:
                    nc.vector.tensor_copy(SHD[:], S_D[:])
                    nc.vector.tensor_copy(S_D[:, 1:QLANES],
                                          SHD[:, 0:QLANES - 1])
                    nc.vector.memset(S_D[:, 0:1], 0.0)
                for s in range(LK):
                    P = p_dpp.tile([52, QLANES], DT)
                    nc.tensor.matmul(P[:], w["wEye"][:],
                                     _cols(UVD[:], s, LK, QLANES),
                                     start=True, stop=False)
                    nc.tensor.matmul(P[:], w["wD"][:], S_D[:],
                                     start=False, stop=True)
                    TD = p_dst.tile([52, QLANES], DT)
                    nc.scalar.activation(TD[:], P[:], AF.Tanh)
                    nc.vector.copy_predicated(
                        S_D[:], _cols(w["u_mcc"][:], s, LK, QLANES), TD[:])
        nc.sync.dma_start(dout["o_SD"][:], S_D[:])

    nc.finalize()
    return nc


# ---------------- entry point ----------------

def kernel(**inputs):
    host = _prep(inputs)
    nc = build_program(host["lq"], host["NLQ"], host["EQ"])

    in_maps = []
    for c in range(NCORES):
        m = {k: np.ascontiguousarray(v) for k, v in host["weights"].items()}
        for name, v in host["cores"][c].items():
            m[name] = np.ascontiguousarray(v)
        in_maps.append(m)

    from concourse.bass_utils import run_bass_kernel_spmd
    res = run_bass_kernel_spmd(nc, in_maps, list(range(NCORES)),
                               trace=TRACE)
    LAST["exec_time_ns"] = res.exec_time_ns
    LAST["results"] = res.results
    LAST["insts"] = res.instructions_and_trace

    K = host["K"]
    g7 = host["geo"][NCORES - 1]
    qs3 = g7["qs"][3]
    r = res.results[NCORES - 1]
    lane = (K - 1 - qs3) // LK + 1
    SD = np.asarray(r["o_SD"])
    hff = np.array([SD[39, lane]], np.float32)
    hfc = SD[40:52, lane].astype(np.float32)
    j = K - 1 - qs3 + EXT
    mean = np.asarray(r["o_mean"])[:, j] + host["bm"]
    lv = np.asarray(r["o_lv"])[:, j] + host["blv"]
    kl = np.float32(-0.5 * np.sum(1.0 + lv - mean * mean - np.exp(lv)))
    return hff, hfc, kl


# revision 13
# speedup vs baseline: 1.5630x; 1.0254x over previous
"""Trainium2 Bass kernel for nn_CHIVE_53111565583018 (clockwork-RNN CHIVE).

The model is a strictly sequential scan (T=131072 encoder steps, K~65536
decoder steps) with tiny (<=32-dim) state, but every chain is strongly
contracting (tanh/gelu + clockwork holds forget initial conditions fast).
We run *chunked* scans: each core processes its time/k range as ~256 chunks
held in the matmul free dimension, advancing all chunks in lockstep, with
2-3 "sweeps" where sweep s re-seeds chunk i with chunk i-1's final state
from sweep s-1.  Boundary error after S sweeps ~ contraction(L)^S, validated
offline (final output rel-err <= ~1e-3).

Per-core phases (8 cores SPMD; core c owns 1/8 of the decoder k-range and an
encoder time window covering it):
  E: encoder frnn/phrnn chains, 64-step chunks x 265 lanes, 2 sweeps;
     records hf+hp per step (via a selection matmul).
  G: repack history to quarter-packed layout (DMA) + indirect_copy gather
     of hf+hp at syllable positions (k-space).
  C: sylrnn hs chain, dense in k-space, 32-step chunks x 4 quarter-chains
     packed on partitions, 2 sweeps; records ys.
  Z: decoder parallel math: mean/logvar/z/hs_new/hp_new/UV projections and
     the cond fill-forward (tensor_tensor_scan).
  D: hff/hfc chain, 32-step chunks, 3 sweeps.
Final outputs (hff[1], hfc[12], kl) assembled on host from core 7's exports.
"""
import os
import sys
import numpy as np
from contextlib import ExitStack

sys.path.insert(0, "/opt/trn_rl_repo")

import concourse.bass as bass
import concourse.bacc as bacc
import concourse.tile as tile
from concourse import mybir

# ---------------- constants (problem-specific, hardcoded) ----------------
T = 131072
H = 32
NCORES = 8

LE = 66            # encoder chunk length (even; lanes*1 matmul = 256 cols)
NRE = 255          # encoder real lanes/core
ELANES = NRE + 1   # + prefix lane = 256
ECOV = NRE * LE    # 16830 encoder steps covered per core
EPRE = LE          # leading prefix columns in encoder buffers
EBUFM = EPRE + ECOV           # mask buffer cols
EBUFX = EPRE + ECOV + 2 * LE  # xfp buffer cols (tail pad for trailing ticks)

LK = 32            # k-space chunk length (C and D chains)
QREAL = 2056       # real k slots per quarter
EXT = 24           # leading real-k extension per quarter
QW = EXT + QREAL   # 2080 recorded k slots per quarter
QWB = 32 + QW      # 2112: + prefix-lane slots (C chain / gather)
NRQ = QW // LK     # 65 real lanes per quarter chain
QLANES = NRQ + 1   # 66
DPAD = LK - EXT    # 8 zero cols ahead of UVD
UVDW = DPAD + QW + EXT  # 2112
XBUF2 = EBUFX // 2 + 32   # 2-block xfp layout cols
MBUF2 = EBUFM // 2        # 2-block mask layout cols

E_SWEEPS = 1
C_SWEEPS = 1
D_SWEEPS = 3

DT = mybir.dt.float32
I8 = mybir.dt.int8
U16 = mybir.dt.uint16
AF = mybir.ActivationFunctionType

TRACE = bool(int(os.environ.get("KERNEL_TRACE", "0")))
DEBUG_OUTS = bool(int(os.environ.get("KERNEL_DEBUG_OUTS", "0")))
JWARM = int(os.environ.get("KERNEL_JWARM", "0"))
LAST = {}  # exec info for the test harness


def _cols(ap2d, start, stride, count):
    return ap2d[:, start: start + (count - 1) * stride + 1: stride]


def _blkdiag4(w):
    n, m = w.shape
    out = np.zeros((4 * n, 4 * m), np.float32)
    for q in range(4):
        out[q * n:(q + 1) * n, q * m:(q + 1) * m] = w
    return out


def _tile4(v):
    return np.tile(np.asarray(v, np.float32), 4)


# ---------------- host-side preprocessing ----------------

def _prep(inputs):
    p = inputs["params"]

    def P(name):
        d = p[name]
        return (np.asarray(d["Wx"], np.float32), np.asarray(d["bx"], np.float32),
                np.asarray(d["Wh"], np.float32), np.asarray(d["bh"], np.float32))

    Wxf0, bxf0, Whf0, bhf0 = P("frnn0")
    Wxf1, bxf1, Whf1, bhf1 = P("frnn1")
    Wxp0, bxp0, Whp0, bhp0 = P("phrnn0")
    Wxp1, bxp1, Whp1, bhp1 = P("phrnn1")
    Wxs, bxs, Wsh, bhs = P("sylrnn")
    Wxd, bxd, _Whd, bhd = P("phrnn_decd")
    Wxff, bxff, Whff, bhff = P("frnn_f")
    Wxfc, bxfc, Whfc, bhfc = P("frnn_c")
    Wm, bm = [np.asarray(x, np.float32) for x in p["bn_mean"]]
    Wlv, blv = [np.asarray(x, np.float32) for x in p["bn_logvar"]]

    xf = np.asarray(inputs["frnn_seq"], np.float32)
    xp = np.asarray(inputs["phrnn_seq"], np.float32)
    xs = np.asarray(inputs["sylrnn_seq"], np.float32)
    eps = np.asarray(inputs["eps"], np.float32)
    cf = np.asarray(inputs["frnn_clock"])
    cp = np.asarray(inputs["phrnn_clock"])
    sf = np.asarray(inputs["sample_freq"])
    dc = np.asarray(inputs["dec_clock"])
    dcc = np.asarray(inputs["dec_clock_c"])

    ts = np.arange(T)
    mf = ((ts % cf) == 0).astype(np.int8)
    mp = ((ts % cp) == 0).astype(np.int8)
    idx = np.nonzero(sf == 1)[0].astype(np.int64)
    K = len(idx)
    i_arr = np.arange(K)
    mcd = ((i_arr % dc[:K]) == 0).astype(np.float32)
    mcc = ((i_arr % dcc[:K]) == 0).astype(np.int8)
    cond = np.zeros(K, np.float32)
    cond[1:] = (sf[:K - 1] == 1).astype(np.float32)

    kb = [round(c * K / NCORES) for c in range(NCORES + 1)]

    # ---- weight blocks (shared) ----
    # W1full: out rows 0:64 = A-state pre-act recurrent part,
    #         rows 64:128 = B pre-act (x-part from A-state + recurrent part)
    wW1 = np.zeros((128, 128), np.float32)
    wW1[0:32, 0:32] = Whf0
    wW1[32:64, 32:64] = Whp0
    wW1[0:32, 64:96] = Wxf1
    wW1[32:64, 96:128] = Wxp1
    wW1[64:96, 64:96] = Whf1
    wW1[96:128, 96:128] = Whp1
    wWX1 = np.zeros((21, 128), np.float32)
    wWX1[0:13, 0:32] = Wxf0
    wWX1[13:20, 32:64] = Wxp0
    wWX1[20, 0:32] = bxf0 + bhf0
    wWX1[20, 32:64] = bxp0 + bhp0
    wWX1[20, 64:96] = bxf1 + bhf1
    wWX1[20, 96:128] = bxp1 + bhp1
    wWX = np.zeros((128, 128), np.float32)   # replicated per 64-row block
    for b in range(2):
        wWX[64 * b:64 * b + 21] = wWX1
    wSel = np.zeros((128, 32), np.float32)   # hf + hp fold for the record
    for i in range(H):
        wSel[64 + i, i] = 1.0
        wSel[96 + i, i] = 1.0

    Wu = np.concatenate([Wxff, Wxfc], axis=1)          # (32,13)
    bu = np.concatenate([bxff + bhff, bxfc + bhfc])    # (13,)
    DD = np.zeros((13, 13), np.float32)
    DD[0, 0] = Whff[0, 0]
    DD[1:, 1:] = Whfc

    weights = {
        "wW1": wW1, "wWX": wWX, "wSel": wSel,
        "wC1": _blkdiag4(Wsh), "wC2": _blkdiag4(Wxs),
        "wMm": _blkdiag4(Wm), "wMlv": _blkdiag4(Wlv),
        "wZ1": _blkdiag4(Wxs), "wZ2": _blkdiag4(Wxd),
        "wU": _blkdiag4(Wu), "wD": _blkdiag4(DD),
        "wEye": np.eye(52, dtype=np.float32),
    }
    bias = np.zeros((128, 8), np.float32)
    bias[:, 0] = _tile4(bxs + bhs)        # C chain / hs_new
    bias[:, 1] = _tile4(bm)               # mean
    bias[:, 2] = _tile4(0.5 * blv)        # 0.5*logvar bias for exp
    bias[:, 3] = _tile4(bxd + bhd)        # hp_new
    bias[0:52, 4] = _tile4(bu)            # UV bias
    weights["bias"] = bias

    # ---- per-core geometry ----
    geo = []
    for c in range(NCORES):
        k0, k1 = kb[c], kb[c + 1]
        assert k1 - k0 <= 4 * QREAL
        qs = [k0 + q * QREAL for q in range(4)]
        kwin0 = max(0, k0 - EXT - 32)
        start = 0 if c == 0 else int(idx[kwin0])
        assert int(idx[k1 - 1]) - start < ECOV, \
            f"core {c}: encoder window too small"
        geo.append({"k0": k0, "k1": k1, "qs": qs, "start": start})

    # uniform per-quarter record-window lane offsets (lane-aligned, shared
    # across cores so the program is SPMD-uniform)
    lq = []
    NLQ = 0
    for q in range(4):
        los, his = [], []
        for c in range(NCORES):
            g = geo[c]
            kq0 = max(0, g["qs"][q] - EXT - 32)
            kqL = min(g["k1"], g["qs"][q] + QREAL) - 1
            los.append(int(idx[kq0]) - g["start"])
            his.append(int(idx[kqL]) - g["start"])
        L_q = max(0, min(los) // LE)
        lq.append(L_q)
        NLQ = max(NLQ, -(-(max(his) + 1 - LE * L_q) // LE))
    NLQ += 1  # margin lane
    assert max(lq) + NLQ <= NRE, f"record window overflow {lq} {NLQ}"
    EQ = NLQ * LE

    # ---- per-core data ----
    cores = []
    for c in range(NCORES):
        g = geo[c]
        k0, k1, qs, start = g["k0"], g["k1"], g["qs"], g["start"]

        tloc = np.arange(EBUFX) - EPRE + start
        ok = (tloc >= 0) & (tloc < T)
        tc_ = np.clip(tloc, 0, T - 1)
        xfp_flat = np.zeros((21, EBUFX), np.float32)
        xfp_flat[0:13] = np.where(ok, xf[tc_].T, 0.0)
        xfp_flat[13:20] = np.where(ok, xp[tc_].T, 0.0)
        xfp_flat[20] = 1.0
        # 2-block layout: flat col j -> block j%2, col j//2
        u_xfp = np.zeros((128, XBUF2), np.float32)
        for b in range(2):
            cols = np.arange(b, EBUFX, 2)
            u_xfp[64 * b:64 * b + 21, 0:len(cols)] = xfp_flat[:, cols]
        msk_flat = np.zeros((64, EBUFM), np.int8)
        okm = ok[:EBUFM]
        tcm = tc_[:EBUFM]
        msk_flat[0:32] = np.where(okm, mf[tcm], 0)
        msk_flat[32:64] = np.where(okm, mp[tcm], 0)
        # 2-block layout (even/odd steps), A rows 0:64 and B rows 64:128
        u_mskE = np.zeros((128, MBUF2), np.int8)
        u_mskO = np.zeros((128, MBUF2), np.int8)
        u_mskE[0:64] = msk_flat[:, 0::2]
        u_mskE[64:128] = u_mskE[0:64]
        u_mskO[0:64] = msk_flat[:, 1::2]
        u_mskO[64:128] = u_mskO[0:64]

        u_gidx = np.zeros((128, QWB // 16), np.uint16)
        u_xsK = np.zeros((128, QWB), np.float32)
        u_eps = np.zeros((128, QW), np.float32)
        u_mcd = np.zeros((128, QW), np.float32)
        u_cnd = np.zeros((52, QW), np.float32)
        u_mcc = np.zeros((52, UVDW), np.int8)
        for q in range(4):
            # gather/C-chain col j (0..QWB) <-> k = qs[q] - EXT - 32 + j
            kk = qs[q] - EXT - 32 + np.arange(QWB)
            live = (kk >= 0) & (kk < k1)
            kkc = np.clip(kk, 0, K - 1)
            tk = np.where(live, idx[kkc] - start - LE * lq[q], 0).astype(np.int64)
            assert tk.min() >= 0 and tk.max() < EQ, f"core {c} q{q} idx range"
            w16 = tk.reshape(QWB // 16, 16).T.astype(np.uint16)
            u_gidx[32 * q:32 * q + 16] = w16
            u_gidx[32 * q + 16:32 * q + 32] = w16
            u_xsK[32 * q:32 * q + 32] = np.where(live, xs[idx[kkc]].T, 0.0)
            # ys/Z col j2 (0..QW) <-> k = qs[q] - EXT + j2
            kk2 = kk[32:]
            live2 = live[32:]
            kkc2 = kkc[32:]
            u_eps[32 * q:32 * q + 32] = np.where(live2, eps[kkc2].T, 0.0)
            u_mcd[32 * q:32 * q + 32] = np.where(live2, mcd[kkc2], 0.0)
            u_cnd[13 * q:13 * q + 13] = np.where(live2, cond[kkc2], 0.0)
            # UVD col j' <-> k = qs[q] + j' - 32
            kk3 = qs[q] + np.arange(UVDW) - 32
            live3 = (kk3 >= 0) & (kk3 < k1) & (np.arange(UVDW) >= DPAD)
            kkc3 = np.clip(kk3, 0, K - 1)
            u_mcc[13 * q, :] = np.where(live3, 1, 0)
            u_mcc[13 * q + 1:13 * q + 13, :] = np.where(live3, mcc[kkc3], 0)

        cores.append({
            "u_xfp": u_xfp, "u_mskE": u_mskE, "u_mskO": u_mskO,
            "u_gidx": u_gidx,
            "u_xsK": u_xsK, "u_eps": u_eps, "u_mcd": u_mcd,
            "u_cnd": u_cnd, "u_mcc": u_mcc,
        })

    return {"K": K, "kb": kb, "bm": bm, "blv": blv, "weights": weights,
            "geo": geo, "cores": cores, "lq": lq, "NLQ": NLQ, "EQ": EQ}


# ---------------- bass program ----------------

def build_program(lq, NLQ, EQ):
    nc = bacc.Bacc()

    din = {}
    for name, shape, dt in [
        ("u_xfp", (128, XBUF2), DT), ("u_mskE", (128, MBUF2), I8),
        ("u_mskO", (128, MBUF2), I8),
        ("u_gidx", (128, QWB // 16), U16),
        ("u_xsK", (128, QWB), DT), ("u_eps", (128, QW), DT),
        ("u_mcd", (128, QW), DT), ("u_cnd", (52, QW), DT),
        ("u_mcc", (52, UVDW), I8),
        ("wW1", (128, 128), DT), ("wWX", (128, 128), DT),
        ("wSel", (128, 32), DT),
        ("wC1", (128, 128), DT), ("wC2", (128, 128), DT),
        ("wMm", (128, 128), DT), ("wMlv", (128, 128), DT),
        ("wZ1", (128, 128), DT), ("wZ2", (128, 128), DT),
        ("wU", (128, 52), DT), ("wD", (52, 52), DT),
        ("wEye", (52, 52), DT), ("bias", (128, 8), DT),
    ]:
        din[name] = nc.declare_dram_parameter(name, list(shape), dt,
                                              isOutput=False)
    dout = {}
    outs = [("o_SD", (52, QLANES), DT), ("o_mean", (32, QW), DT),
            ("o_lv", (32, QW), DT)]
    if DEBUG_OUTS:
        outs += [("o_hq", (128, EQ), DT), ("o_ys", (128, QW), DT),
                 ("o_uvd", (52, UVDW), DT), ("o_resk", (128, QWB), DT)]
    for name, shape, dt in outs:
        dout[name] = nc.declare_dram_parameter(name, list(shape), dt,
                                               isOutput=True)

    with tile.TileContext(nc) as tc, ExitStack() as ctx:
        pc = ctx.enter_context(tc.tile_pool(name="const", bufs=1))
        po = ctx.enter_context(tc.tile_pool(name="outer", bufs=1))

        w = {}
        for name in ["wW1", "wWX", "wSel", "wC1", "wC2", "wMm", "wMlv",
                     "wZ1", "wZ2", "wU", "wD", "wEye", "bias", "u_gidx",
                     "u_xsK", "u_eps", "u_mcd", "u_cnd", "u_mcc"]:
            t = pc.tile(list(din[name].shape), din[name].dtype, tag=name)
            nc.gpsimd.dma_start(t[:], din[name][:])
            w[name] = t
        bias = w["bias"]

        # engine warm-ups: let each engine observe the DMA sems on a cheap op
        wrm = pc.tile([128, 8], DT)
        nc.vector.tensor_copy(wrm[0:52, 0:1], w["u_cnd"][:, 0:1])
        nc.vector.tensor_copy(wrm[:, 1:2], w["u_mcd"][:, 0:1])
        nc.vector.tensor_copy(wrm[:, 2:3], w["u_eps"][:, 0:1])
        nc.vector.tensor_copy(wrm[:, 3:4], w["u_xsK"][:, 0:1])
        wrm8 = pc.tile([128, 2], I8)
        nc.vector.tensor_copy(wrm8[0:52, 0:1], w["u_mcc"][:, 0:1])
        wrm16 = pc.tile([128, 1], U16)
        nc.vector.tensor_copy(wrm16[:], w["u_gidx"][:, 0:1])
        # dummy gather: forces the gpsimd ap_gather library load early so it
        # overlaps phase E instead of blocking phase G
        zidx = pc.tile([128, 16], U16)
        nc.vector.memset(zidx[:], 0)
        gjunk = pc.tile([128, 256], DT)
        nc.vector.memset(gjunk[:], 0.0)
        nc.gpsimd.indirect_copy(gjunk[:, 0:16], gjunk[:], zidx[:, 0:1], True)

        # persistent k-space tiles
        ysK = po.tile([128, QW], DT)
        RESK = po.tile([128, QWB], DT)
        UV52 = po.tile([52, QW], DT)
        UVD = po.tile([52, UVDW], DT)

        # ---------------- phase E ----------------
        S128 = po.tile([128, ELANES], DT)   # rows 0:64 A, rows 64:128 B
        SHT = po.tile([128, ELANES], DT)
        nc.vector.memset(S128[:], 0.0)

        with tc.tile_pool(name="hq", bufs=1) as p_hq:
            HQR = p_hq.tile([128, EQ], DT)   # quarter-windowed hf+hp history
            with tc.tile_pool(name="ein", bufs=1) as p_ein, \
                 tc.tile_pool(name="est", bufs=3) as p_est, \
                 tc.tile_pool(name="e_ps", bufs=3, space="PSUM") as p_epp, \
                 tc.tile_pool(name="e_psr", bufs=3, space="PSUM") as p_epr, \
                 tc.tile_pool(name="e_psj", bufs=2, space="PSUM") as p_epj:
                xfp = p_ein.tile([128, XBUF2], DT)
                mskE = p_ein.tile([128, MBUF2], I8)
                mskO = p_ein.tile([128, MBUF2], I8)
                nc.gpsimd.dma_start(xfp[:], din["u_xfp"][:])
                nc.gpsimd.dma_start(mskE[:], din["u_mskE"][:])
                nc.gpsimd.dma_start(mskO[:], din["u_mskO"][:])
                nc.vector.tensor_copy(wrm8[:, 1:2], mskE[:, 0:1])
                nc.vector.tensor_copy(wrm8[0:64, 0:1], mskO[0:64, 0:1])

                def mska(step):
                    t = mskE if step % 2 == 0 else mskO
                    return _cols(t[0:64, :], step // 2, LE // 2, ELANES)

                def mskb(step):
                    t = mskE if step % 2 == 0 else mskO
                    return _cols(t[64:128, :], step // 2, LE // 2, ELANES)

                for sweep in range(E_SWEEPS):
                    if sweep:
                        nc.vector.tensor_copy(SHT[:], S128[:])
                        nc.vector.tensor_copy(S128[:, 1:ELANES],
                                              SHT[:, 0:ELANES - 1])
                        nc.vector.memset(S128[:, 0:1], 0.0)
                    for tk in range(LE + 2):
                        a_on = tk < LE
                        b_on = 1 <= tk <= LE
                        r_on = sweep == E_SWEEPS - 1 and 2 <= tk
                        if a_on or b_on:
                            P = p_epp.tile([128, ELANES], DT)
                            b2 = 64 * (tk % 2)
                            nc.tensor.matmul(P[:], w["wWX"][b2:b2 + 21, :],
                                             _cols(xfp[b2:b2 + 21, :],
                                                   tk // 2, LE // 2, ELANES),
                                             start=True, stop=False)
                            nc.tensor.matmul(P[:], w["wW1"][:], S128[:],
                                             start=False, stop=True)
                        if r_on:
                            # record hf+hp of step tk-2: S128 B-rows still
                            # hold that state until this tick's cpB
                            sR = tk - 2
                            PR = p_epr.tile([32, ELANES], DT)
                            nc.tensor.matmul(PR[:], w["wSel"][:], S128[:],
                                             start=True, stop=True)
                            for q in range(4):
                                nc.vector.tensor_copy(
                                    _cols(HQR[32 * q:32 * q + 32, :],
                                          sR, LE, NLQ),
                                    PR[:, lq[q] + 1:lq[q] + NLQ + 1])
                        if a_on or b_on:
                            T1 = p_est.tile([128, ELANES], DT)
                            nc.scalar.activation(T1[:], P[:], AF.Tanh)
                        if b_on:
                            sB = tk - 1
                            nc.vector.copy_predicated(
                                S128[64:128, :], mskb(sB), T1[64:128, :])
                        if a_on:
                            nc.vector.copy_predicated(
                                S128[0:64, :], mska(tk), T1[0:64, :])
                        if b_on:
                            nc.scalar.activation(S128[64:128, :],
                                                 S128[64:128, :], AF.Gelu)
                        for jw in range(JWARM):
                            PJ = p_epj.tile([128, 128], DT)
                            nc.tensor.matmul(PJ[:], w["wW1"][:],
                                             xfp[:, 128 * jw:128 * jw + 128],
                                             start=True, stop=True)
            if DEBUG_OUTS:
                nc.sync.dma_start(dout["o_hq"][:], HQR[:])

            # ---------------- phase G ----------------
            with tc.tile_pool(name="gat", bufs=1) as p_g:
                HFPK = p_g.tile([128, QWB], DT)
                nc.vector.tensor_copy(wrm[:, 4:5], HQR[:, 0:1])
                NIC = 352  # indirect_copy output-width ISA cap is ~448
                for o in range(0, QWB, NIC):
                    nc.gpsimd.indirect_copy(
                        HFPK[:, o:o + NIC], HQR[:],
                        w["u_gidx"][:, o // 16:(o + NIC) // 16], True)
                nc.vector.tensor_add(RESK[:], HFPK[:], w["u_xsK"][:])
        if DEBUG_OUTS:
            nc.sync.dma_start(dout["o_resk"][:], RESK[:])

        # ---------------- phase C ----------------
        S_C = po.tile([128, QLANES], DT)
        nc.vector.memset(S_C[:], 0.0)
        SHC = po.tile([128, QLANES], DT)
        with tc.tile_pool(name="cst", bufs=3) as p_cst, \
             tc.tile_pool(name="c_ps", bufs=3, space="PSUM") as p_cpp:
            for sweep in range(C_SWEEPS):
                if sweep:
                    nc.vector.tensor_copy(SHC[:], S_C[:])
                    nc.vector.tensor_copy(S_C[:, 1:QLANES],
                                          SHC[:, 0:QLANES - 1])
                    nc.vector.memset(S_C[:, 0:1], 0.0)
                for s in range(LK):
                    P = p_cpp.tile([128, QLANES], DT)
                    nc.tensor.matmul(P[:], w["wC2"][:],
                                     _cols(RESK[:], s, LK, QLANES),
                                     start=True, stop=False)
                    nc.tensor.matmul(P[:], w["wC1"][:], S_C[:],
                                     start=False, stop=True)
                    TC = p_cst.tile([128, QLANES], DT)
                    nc.scalar.activation(TC[:], P[:], AF.Tanh,
                                         bias=bias[:, 0:1])
                    nc.scalar.activation(S_C[:], TC[:], AF.Gelu)
                    if sweep == C_SWEEPS - 1:
                        nc.vector.tensor_copy(_cols(ysK[:], s, LK, NRQ),
                                              S_C[:, 1:QLANES])
        if DEBUG_OUTS:
            nc.sync.dma_start(dout["o_ys"][:], ysK[:])

        # ---------------- phase Z ----------------
        SL = [(i, min(512, QW - i)) for i in range(0, QW, 512)]
        with tc.tile_pool(name="zbuf", bufs=1) as p_z, \
             tc.tile_pool(name="z_ps", bufs=4, space="PSUM") as p_zpp:
            MEAN = p_z.tile([128, QW], DT)
            LV = p_z.tile([128, QW], DT)
            ET = p_z.tile([128, QW], DT)
            PZ = p_z.tile([128, QW], DT)
            ZZ = p_z.tile([128, QW], DT)
            TS = p_z.tile([128, QW], DT)
            GS = p_z.tile([128, QW], DT)
            HSN = p_z.tile([128, QW], DT)
            HPN = p_z.tile([128, QW], DT)

            for dst, wm in ((MEAN, "wMm"), (LV, "wMlv")):
                for o, n in SL:
                    PM = p_zpp.tile([128, 512], DT)
                    nc.tensor.matmul(PM[:, 0:n], w[wm][:], ysK[:, o:o + n],
                                     start=True, stop=True)
                    nc.vector.tensor_copy(dst[:, o:o + n], PM[:, 0:n])
            nc.sync.dma_start(dout["o_mean"][:], MEAN[96:128, :])
            nc.sync.dma_start(dout["o_lv"][:], LV[96:128, :])
            nc.scalar.activation(ET[:], LV[:], AF.Exp,
                                 bias=bias[:, 2:3], scale=0.5)
            nc.vector.tensor_mul(PZ[:], ET[:], w["u_eps"][:])
            nc.vector.scalar_tensor_tensor(
                ZZ[:], MEAN[:], bias[:, 1:2], PZ[:],
                op0=mybir.AluOpType.add, op1=mybir.AluOpType.add)
            for o, n in SL:
                PM = p_zpp.tile([128, 512], DT)
                nc.tensor.matmul(PM[:, 0:n], w["wZ1"][:], ZZ[:, o:o + n],
                                 start=True, stop=True)
                nc.scalar.activation(TS[:, o:o + n], PM[:, 0:n], AF.Tanh,
                                     bias=bias[:, 0:1])
            nc.scalar.activation(GS[:], TS[:], AF.Gelu)
            nc.vector.tensor_mul(HSN[:], GS[:], w["u_mcd"][:])
            for o, n in SL:
                PM = p_zpp.tile([128, 512], DT)
                nc.tensor.matmul(PM[:, 0:n], w["wZ2"][:], HSN[:, o:o + n],
                                 start=True, stop=True)
                nc.scalar.activation(TS[:, o:o + n], PM[:, 0:n], AF.Tanh,
                                     bias=bias[:, 3:4])
            nc.scalar.activation(GS[:], TS[:], AF.Gelu)
            nc.vector.tensor_mul(HPN[:], GS[:], w["u_mcd"][:])
            for o, n in SL:
                PU = p_zpp.tile([128, 512], DT)
                nc.tensor.matmul(PU[0:52, 0:n], w["wU"][:], HPN[:, o:o + n],
                                 start=True, stop=True)
                nc.scalar.activation(UV52[:, o:o + n], PU[0:52, 0:n],
                                     AF.Identity, bias=bias[0:52, 4:5])
            CM = p_z.tile([52, QW], DT)
            CUV = p_z.tile([52, QW], DT)
            nc.vector.tensor_scalar(CM[:], w["u_cnd"][:], -1.0, 1.0,
                                    op0=mybir.AluOpType.mult,
                                    op1=mybir.AluOpType.add)
            nc.vector.tensor_mul(CUV[:], UV52[:], w["u_cnd"][:])
            nc.vector.memset(UVD[:, 0:DPAD], 0.0)
            nc.vector.memset(UVD[:, DPAD + QW:], 0.0)
            nc.vector.tensor_tensor_scan(
                UVD[:, DPAD:DPAD + QW], CM[:], CUV[:], 0.0,
                op0=mybir.AluOpType.mult, op1=mybir.AluOpType.add)
        if DEBUG_OUTS:
            nc.sync.dma_start(dout["o_uvd"][:], UVD[:])

        # ---------------- phase D ----------------
        S_D = po.tile([52, QLANES], DT)
        nc.vector.memset(S_D[:], 0.0)
        SHD = po.tile([52, QLANES], DT)
        with tc.tile_pool(name="dst", bufs=3) as p_dst, \
             tc.tile_pool(name="d_ps", bufs=3, space="PSUM") as p_dpp:
            for sweep in range(D_SWEEPS):
                if sweep:
                    nc.vector.tensor_copy(SHD[:], S_D[:])
                    nc.vector.tensor_copy(S_D[:, 1:QLANES],
                                          SHD[:, 0:QLANES - 1])
                    nc.vector.memset(S_D[:, 0:1], 0.0)
                for s in range(LK):
                    P = p_dpp.tile([52, QLANES], DT)
                    nc.tensor.matmul(P[:], w["wEye"][:],
                                     _cols(UVD[:], s, LK, QLANES),
                                     start=True, stop=False)
                    nc.tensor.matmul(P[:], w["wD"][:], S_D[:],
                                     start=False, stop=True)
                    TD = p_dst.tile([52, QLANES], DT)
                    nc.scalar.activation(TD[:], P[:], AF.Tanh)
                    nc.vector.copy_predicated(
                        S_D[:], _cols(w["u_mcc"][:], s, LK, QLANES), TD[:])
        nc.sync.dma_start(dout["o_SD"][:], S_D[:])

    nc.finalize()
    return nc


# ---------------- entry point ----------------

def kernel(**inputs):
    host = _prep(inputs)
    nc = build_program(host["lq"], host["NLQ"], host["EQ"])

    in_maps = []
    for c in range(NCORES):
        m = {k: np.ascontiguousarray(v) for k, v in host["weights"].items()}
        for name, v in host["cores"][c].items():
            m[name] = np.ascontiguousarray(v)
        in_maps.append(m)

    from concourse.bass_utils import run_bass_kernel_spmd
    res = run_bass_kernel_spmd(nc, in_maps, list(range(NCORES)),
                               trace=TRACE)
    LAST["exec_time_ns"] = res.exec_time_ns
    LAST["results"] = res.results
    LAST["insts"] = res.instructions_and_trace

    K = host["K"]
    g7 = host["geo"][NCORES - 1]
    qs3 = g7["qs"][3]
    r = res.results[NCORES - 1]
    lane = (K - 1 - qs3) // LK + 1
    SD = np.asarray(r["o_SD"])
    hff = np.array([SD[39, lane]], np.float32)
    hfc = SD[40:52, lane].astype(np.float32)
    j = K - 1 - qs3 + EXT
    mean = np.asarray(r["o_mean"])[:, j] + host["bm"]
    lv = np.asarray(r["o_lv"])[:, j] + host["blv"]
    kl = np.float32(-0.5 * np.sum(1.0 + lv - mean * mean - np.exp(lv)))
    return hff, hfc, kl


# revision 15
# speedup vs baseline: 1.6367x; 1.0472x over previous
"""Trainium2 Bass kernel for nn_CHIVE_53111565583018 (clockwork-RNN CHIVE).

The model is a strictly sequential scan (T=131072 encoder steps, K~65536
decoder steps) with tiny (<=32-dim) state, but every chain is strongly
contracting (tanh/gelu + clockwork holds forget initial conditions fast).
We run *chunked* scans: each core processes its time/k range as ~256 chunks
held in the matmul free dimension, advancing all chunks in lockstep, with
2-3 "sweeps" where sweep s re-seeds chunk i with chunk i-1's final state
from sweep s-1.  Boundary error after S sweeps ~ contraction(L)^S, validated
offline (final output rel-err <= ~1e-3).

Per-core phases (8 cores SPMD; core c owns 1/8 of the decoder k-range and an
encoder time window covering it):
  E: encoder frnn/phrnn chains, 64-step chunks x 265 lanes, 2 sweeps;
     records hf+hp per step (via a selection matmul).
  G: repack history to quarter-packed layout (DMA) + indirect_copy gather
     of hf+hp at syllable positions (k-space).
  C: sylrnn hs chain, dense in k-space, 32-step chunks x 4 quarter-chains
     packed on partitions, 2 sweeps; records ys.
  Z: decoder parallel math: mean/logvar/z/hs_new/hp_new/UV projections and
     the cond fill-forward (tensor_tensor_scan).
  D: hff/hfc chain, 32-step chunks, 3 sweeps.
Final outputs (hff[1], hfc[12], kl) assembled on host from core 7's exports.
"""
import os
import sys
import numpy as np
from contextlib import ExitStack

sys.path.insert(0, "/opt/trn_rl_repo")

import concourse.bass as bass
import concourse.bacc as bacc
import concourse.tile as tile
from concourse import mybir

# ---------------- constants (problem-specific, hardcoded) ----------------
T = 131072
H = 32
NCORES = 8

LE = 66            # encoder chunk length (even; lanes*1 matmul = 256 cols)
NRE = 255          # encoder real lanes/core
ELANES = NRE + 1   # + prefix lane = 256
ECOV = NRE * LE    # 16830 encoder steps covered per core
EPRE = LE          # leading prefix columns in encoder buffers
EBUFM = EPRE + ECOV + 2       # mask buffer cols (+2: tick-LE reads)
EBUFX = EPRE + ECOV + 2 * LE  # xfp buffer cols (tail pad for trailing ticks)

LK = 32            # k-space chunk length (C and D chains)
QREAL = 2056       # real k slots per quarter
EXT = 24           # leading real-k extension per quarter
QW = EXT + QREAL   # 2080 recorded k slots per quarter
QWB = 32 + QW      # 2112: + prefix-lane slots (C chain / gather)
NRQ = QW // LK     # 65 real lanes per quarter chain
QLANES = NRQ + 1   # 66
DPAD = LK - EXT    # 8 zero cols ahead of UVD
UVDW = DPAD + QW + EXT  # 2112
XBUF2 = EBUFX // 2 + 32   # 2-block xfp layout cols
MBUF2 = EBUFM // 2        # 2-block mask layout cols

E_SWEEPS = 1
C_SWEEPS = 1
D_SWEEPS = 3

DT = mybir.dt.float32
I8 = mybir.dt.int8
U16 = mybir.dt.uint16
AF = mybir.ActivationFunctionType

TRACE = bool(int(os.environ.get("KERNEL_TRACE", "0")))
DEBUG_OUTS = bool(int(os.environ.get("KERNEL_DEBUG_OUTS", "0")))
JWARM = int(os.environ.get("KERNEL_JWARM", "0"))
LAST = {}  # exec info for the test harness


def _cols(ap2d, start, stride, count):
    return ap2d[:, start: start + (count - 1) * stride + 1: stride]


def _blkdiag4(w):
    n, m = w.shape
    out = np.zeros((4 * n, 4 * m), np.float32)
    for q in range(4):
        out[q * n:(q + 1) * n, q * m:(q + 1) * m] = w
    return out


def _tile4(v):
    return np.tile(np.asarray(v, np.float32), 4)


# ---------------- host-side preprocessing ----------------

def _prep(inputs):
    p = inputs["params"]

    def P(name):
        d = p[name]
        return (np.asarray(d["Wx"], np.float32), np.asarray(d["bx"], np.float32),
                np.asarray(d["Wh"], np.float32), np.asarray(d["bh"], np.float32))

    Wxf0, bxf0, Whf0, bhf0 = P("frnn0")
    Wxf1, bxf1, Whf1, bhf1 = P("frnn1")
    Wxp0, bxp0, Whp0, bhp0 = P("phrnn0")
    Wxp1, bxp1, Whp1, bhp1 = P("phrnn1")
    Wxs, bxs, Wsh, bhs = P("sylrnn")
    Wxd, bxd, _Whd, bhd = P("phrnn_decd")
    Wxff, bxff, Whff, bhff = P("frnn_f")
    Wxfc, bxfc, Whfc, bhfc = P("frnn_c")
    Wm, bm = [np.asarray(x, np.float32) for x in p["bn_mean"]]
    Wlv, blv = [np.asarray(x, np.float32) for x in p["bn_logvar"]]

    xf = np.asarray(inputs["frnn_seq"], np.float32)
    xp = np.asarray(inputs["phrnn_seq"], np.float32)
    xs = np.asarray(inputs["sylrnn_seq"], np.float32)
    eps = np.asarray(inputs["eps"], np.float32)
    cf = np.asarray(inputs["frnn_clock"])
    cp = np.asarray(inputs["phrnn_clock"])
    sf = np.asarray(inputs["sample_freq"])
    dc = np.asarray(inputs["dec_clock"])
    dcc = np.asarray(inputs["dec_clock_c"])

    ts = np.arange(T)
    mf = ((ts % cf) == 0).astype(np.int8)
    mp = ((ts % cp) == 0).astype(np.int8)
    idx = np.nonzero(sf == 1)[0].astype(np.int64)
    K = len(idx)
    i_arr = np.arange(K)
    mcd = ((i_arr % dc[:K]) == 0).astype(np.float32)
    mcc = ((i_arr % dcc[:K]) == 0).astype(np.int8)
    cond = np.zeros(K, np.float32)
    cond[1:] = (sf[:K - 1] == 1).astype(np.float32)

    kb = [round(c * K / NCORES) for c in range(NCORES + 1)]

    # ---- weight blocks (shared) ----
    # W1full: out rows 0:64 = A-state pre-act recurrent part,
    #         rows 64:128 = B pre-act (x-part from A-state + recurrent part)
    wW1 = np.zeros((128, 128), np.float32)
    wW1[0:32, 0:32] = Whf0
    wW1[32:64, 32:64] = Whp0
    wW1[0:32, 64:96] = Wxf1
    wW1[32:64, 96:128] = Wxp1
    wW1[64:96, 64:96] = Whf1
    wW1[96:128, 96:128] = Whp1
    wWX1 = np.zeros((21, 128), np.float32)
    wWX1[0:13, 0:32] = Wxf0
    wWX1[13:20, 32:64] = Wxp0
    wWX1[20, 0:32] = bxf0 + bhf0
    wWX1[20, 32:64] = bxp0 + bhp0
    wWX1[20, 64:96] = bxf1 + bhf1
    wWX1[20, 96:128] = bxp1 + bhp1
    wWX = np.zeros((128, 128), np.float32)   # replicated per 64-row block
    for b in range(2):
        wWX[64 * b:64 * b + 21] = wWX1
    wSel = np.zeros((128, 32), np.float32)   # hf + hp fold for the record
    for i in range(H):
        wSel[64 + i, i] = 1.0
        wSel[96 + i, i] = 1.0

    Wu = np.concatenate([Wxff, Wxfc], axis=1)          # (32,13)
    bu = np.concatenate([bxff + bhff, bxfc + bhfc])    # (13,)
    DD = np.zeros((13, 13), np.float32)
    DD[0, 0] = Whff[0, 0]
    DD[1:, 1:] = Whfc

    weights = {
        "wW1": wW1, "wWX": wWX, "wSel": wSel,
        "wC1": _blkdiag4(Wsh), "wC2": _blkdiag4(Wxs),
        "wMm": _blkdiag4(Wm), "wMlv": _blkdiag4(Wlv),
        "wZ1": _blkdiag4(Wxs), "wZ2": _blkdiag4(Wxd),
        "wU": _blkdiag4(Wu), "wD": _blkdiag4(DD),
        "wEye": np.eye(52, dtype=np.float32),
    }
    bias = np.zeros((128, 8), np.float32)
    bias[:, 0] = _tile4(bxs + bhs)        # C chain / hs_new
    bias[:, 1] = _tile4(bm)               # mean
    bias[:, 2] = _tile4(0.5 * blv)        # 0.5*logvar bias for exp
    bias[:, 3] = _tile4(bxd + bhd)        # hp_new
    bias[0:52, 4] = _tile4(bu)            # UV bias
    weights["bias"] = bias

    # ---- per-core geometry ----
    geo = []
    for c in range(NCORES):
        k0, k1 = kb[c], kb[c + 1]
        assert k1 - k0 <= 4 * QREAL
        qs = [k0 + q * QREAL for q in range(4)]
        kwin0 = max(0, k0 - EXT - 32)
        start = 0 if c == 0 else int(idx[kwin0])
        assert int(idx[k1 - 1]) - start < ECOV, \
            f"core {c}: encoder window too small"
        geo.append({"k0": k0, "k1": k1, "qs": qs, "start": start})

    # uniform per-quarter record-window lane offsets (lane-aligned, shared
    # across cores so the program is SPMD-uniform)
    lq = []
    NLQ = 0
    for q in range(4):
        los, his = [], []
        for c in range(NCORES):
            g = geo[c]
            kq0 = max(0, g["qs"][q] - EXT - 32)
            kqL = min(g["k1"], g["qs"][q] + QREAL) - 1
            los.append(int(idx[kq0]) - g["start"])
            his.append(int(idx[kqL]) - g["start"])
        L_q = max(0, min(los) // LE)
        lq.append(L_q)
        NLQ = max(NLQ, -(-(max(his) + 1 - LE * L_q) // LE))
    NLQ += 1  # margin lane
    assert max(lq) + NLQ <= NRE, f"record window overflow {lq} {NLQ}"
    EQ = NLQ * LE

    # ---- per-core data ----
    cores = []
    for c in range(NCORES):
        g = geo[c]
        k0, k1, qs, start = g["k0"], g["k1"], g["qs"], g["start"]

        tloc = np.arange(EBUFX) - EPRE + start
        ok = (tloc >= 0) & (tloc < T)
        tc_ = np.clip(tloc, 0, T - 1)
        xfp_flat = np.zeros((21, EBUFX), np.float32)
        xfp_flat[0:13] = np.where(ok, xf[tc_].T, 0.0)
        xfp_flat[13:20] = np.where(ok, xp[tc_].T, 0.0)
        xfp_flat[20] = 1.0
        # 2-block layout: flat col j -> block j%2, col j//2
        u_xfp = np.zeros((128, XBUF2), np.float32)
        for b in range(2):
            cols = np.arange(b, EBUFX, 2)
            u_xfp[64 * b:64 * b + 21, 0:len(cols)] = xfp_flat[:, cols]
        msk_flat = np.zeros((64, EBUFM), np.int8)
        okm = ok[:EBUFM]
        tcm = tc_[:EBUFM]
        msk_flat[0:32] = np.where(okm, mf[tcm], 0)
        msk_flat[32:64] = np.where(okm, mp[tcm], 0)
        msk_flat[:, EBUFM - 2:] = 0
        # 2-block layout (even/odd steps), A rows 0:64 and B rows 64:128
        u_mskE = np.zeros((128, MBUF2), np.int8)
        u_mskO = np.zeros((128, MBUF2), np.int8)
        u_mskE[0:64] = msk_flat[:, 0::2]
        # B-half masks are for the PREVIOUS step (B lags A by one tick)
        u_mskE[64:128, 1:] = msk_flat[:, 1:-1:2]
        u_mskO[0:64] = msk_flat[:, 1::2]
        u_mskO[64:128] = msk_flat[:, 0::2]

        u_gidx = np.zeros((128, QWB // 16), np.uint16)
        u_xsK = np.zeros((128, QWB), np.float32)
        u_eps = np.zeros((128, QW), np.float32)
        u_mcd = np.zeros((128, QW), np.float32)
        u_cnd = np.zeros((52, QW), np.float32)
        u_mcc = np.zeros((52, UVDW), np.int8)
        for q in range(4):
            # gather/C-chain col j (0..QWB) <-> k = qs[q] - EXT - 32 + j
            kk = qs[q] - EXT - 32 + np.arange(QWB)
            live = (kk >= 0) & (kk < k1)
            kkc = np.clip(kk, 0, K - 1)
            tk = np.where(live, idx[kkc] - start - LE * lq[q], 0).astype(np.int64)
            assert tk.min() >= 0 and tk.max() < EQ, f"core {c} q{q} idx range"
            w16 = tk.reshape(QWB // 16, 16).T.astype(np.uint16)
            u_gidx[32 * q:32 * q + 16] = w16
            u_gidx[32 * q + 16:32 * q + 32] = w16
            u_xsK[32 * q:32 * q + 32] = np.where(live, xs[idx[kkc]].T, 0.0)
            # ys/Z col j2 (0..QW) <-> k = qs[q] - EXT + j2
            kk2 = kk[32:]
            live2 = live[32:]
            kkc2 = kkc[32:]
            u_eps[32 * q:32 * q + 32] = np.where(live2, eps[kkc2].T, 0.0)
            u_mcd[32 * q:32 * q + 32] = np.where(live2, mcd[kkc2], 0.0)
            u_cnd[13 * q:13 * q + 13] = np.where(live2, cond[kkc2], 0.0)
            # UVD col j' <-> k = qs[q] + j' - 32
            kk3 = qs[q] + np.arange(UVDW) - 32
            live3 = (kk3 >= 0) & (kk3 < k1) & (np.arange(UVDW) >= DPAD)
            kkc3 = np.clip(kk3, 0, K - 1)
            u_mcc[13 * q, :] = np.where(live3, 1, 0)
            u_mcc[13 * q + 1:13 * q + 13, :] = np.where(live3, mcc[kkc3], 0)

        cores.append({
            "u_xfp": u_xfp, "u_mskE": u_mskE, "u_mskO": u_mskO,
            "u_gidx": u_gidx,
            "u_xsK": u_xsK, "u_eps": u_eps, "u_mcd": u_mcd,
            "u_cnd": u_cnd, "u_mcc": u_mcc,
        })

    return {"K": K, "kb": kb, "bm": bm, "blv": blv, "weights": weights,
            "geo": geo, "cores": cores, "lq": lq, "NLQ": NLQ, "EQ": EQ}


# ---------------- bass program ----------------

def build_program(lq, NLQ, EQ):
    nc = bacc.Bacc()

    din = {}
    for name, shape, dt in [
        ("u_xfp", (128, XBUF2), DT), ("u_mskE", (128, MBUF2), I8),
        ("u_mskO", (128, MBUF2), I8),
        ("u_gidx", (128, QWB // 16), U16),
        ("u_xsK", (128, QWB), DT), ("u_eps", (128, QW), DT),
        ("u_mcd", (128, QW), DT), ("u_cnd", (52, QW), DT),
        ("u_mcc", (52, UVDW), I8),
        ("wW1", (128, 128), DT), ("wWX", (128, 128), DT),
        ("wSel", (128, 32), DT),
        ("wC1", (128, 128), DT), ("wC2", (128, 128), DT),
        ("wMm", (128, 128), DT), ("wMlv", (128, 128), DT),
        ("wZ1", (128, 128), DT), ("wZ2", (128, 128), DT),
        ("wU", (128, 52), DT), ("wD", (52, 52), DT),
        ("wEye", (52, 52), DT), ("bias", (128, 8), DT),
    ]:
        din[name] = nc.declare_dram_parameter(name, list(shape), dt,
                                              isOutput=False)
    dout = {}
    outs = [("o_SD", (52, QLANES), DT), ("o_mean", (32, QW), DT),
            ("o_lv", (32, QW), DT)]
    if DEBUG_OUTS:
        outs += [("o_hq", (128, EQ), DT), ("o_ys", (128, QW), DT),
                 ("o_uvd", (52, UVDW), DT), ("o_resk", (128, QWB), DT)]
    for name, shape, dt in outs:
        dout[name] = nc.declare_dram_parameter(name, list(shape), dt,
                                               isOutput=True)

    with tile.TileContext(nc) as tc, ExitStack() as ctx:
        pc = ctx.enter_context(tc.tile_pool(name="const", bufs=1))
        po = ctx.enter_context(tc.tile_pool(name="outer", bufs=1))

        w = {}
        for name in ["wW1", "wWX", "wSel", "wC1", "wC2", "wMm", "wMlv",
                     "wZ1", "wZ2", "wU", "wD", "wEye", "bias", "u_gidx",
                     "u_xsK", "u_eps", "u_mcd", "u_cnd", "u_mcc"]:
            t = pc.tile(list(din[name].shape), din[name].dtype, tag=name)
            nc.sync.dma_start(t[:], din[name][:])
            w[name] = t
        bias = w["bias"]

        # engine warm-ups: let each engine observe the DMA sems on a cheap op
        wrm = pc.tile([128, 8], DT)
        nc.vector.tensor_copy(wrm[0:52, 0:1], w["u_cnd"][:, 0:1])
        nc.vector.tensor_copy(wrm[:, 1:2], w["u_mcd"][:, 0:1])
        nc.vector.tensor_copy(wrm[:, 2:3], w["u_eps"][:, 0:1])
        nc.vector.tensor_copy(wrm[:, 3:4], w["u_xsK"][:, 0:1])
        wrm8 = pc.tile([128, 2], I8)
        nc.vector.tensor_copy(wrm8[0:52, 0:1], w["u_mcc"][:, 0:1])
        wrm16 = pc.tile([128, 1], U16)
        nc.vector.tensor_copy(wrm16[:], w["u_gidx"][:, 0:1])
        # dummy gather: forces the gpsimd ap_gather library load early so it
        # overlaps phase E instead of blocking phase G
        zidx = pc.tile([128, 16], U16)
        nc.vector.memset(zidx[:], 0)
        gjunk = pc.tile([128, 256], DT)
        nc.vector.memset(gjunk[:], 0.0)
        nc.gpsimd.indirect_copy(gjunk[:, 0:16], gjunk[:], zidx[:, 0:1], True)

        # persistent k-space tiles
        ysK = po.tile([128, QW], DT)
        RESK = po.tile([128, QWB], DT)
        UV52 = po.tile([52, QW], DT)
        UVD = po.tile([52, UVDW], DT)

        # ---------------- phase E ----------------
        S128 = po.tile([128, ELANES], DT)   # rows 0:64 A, rows 64:128 B
        SHT = po.tile([128, ELANES], DT)
        nc.vector.memset(S128[:], 0.0)

        with tc.tile_pool(name="hq", bufs=1) as p_hq:
            HQR = p_hq.tile([128, EQ], DT)   # quarter-windowed hf+hp history
            with tc.tile_pool(name="ein", bufs=1) as p_ein, \
                 tc.tile_pool(name="est", bufs=3) as p_est, \
                 tc.tile_pool(name="e_ps", bufs=3, space="PSUM") as p_epp, \
                 tc.tile_pool(name="e_psr", bufs=3, space="PSUM") as p_epr, \
                 tc.tile_pool(name="e_psj", bufs=2, space="PSUM") as p_epj:
                xfp = p_ein.tile([128, XBUF2], DT)
                mskE = p_ein.tile([128, MBUF2], I8)
                mskO = p_ein.tile([128, MBUF2], I8)
                nc.sync.dma_start(xfp[:], din["u_xfp"][:])
                nc.sync.dma_start(mskE[:], din["u_mskE"][:])
                nc.sync.dma_start(mskO[:], din["u_mskO"][:])
                nc.vector.tensor_copy(wrm8[:, 1:2], mskE[:, 0:1])
                nc.vector.tensor_copy(wrm8[0:64, 0:1], mskO[0:64, 0:1])

                def mskab(step):
                    t = mskE if step % 2 == 0 else mskO
                    return _cols(t[:, :], step // 2, LE // 2, ELANES)

                for sweep in range(E_SWEEPS):
                    if sweep:
                        nc.vector.tensor_copy(SHT[:], S128[:])
                        nc.vector.tensor_copy(S128[:, 1:ELANES],
                                              SHT[:, 0:ELANES - 1])
                        nc.vector.memset(S128[:, 0:1], 0.0)
                    for tk in range(LE + 2):
                        a_on = tk < LE
                        b_on = 1 <= tk <= LE
                        r_on = sweep == E_SWEEPS - 1 and 2 <= tk
                        if a_on or b_on:
                            P = p_epp.tile([128, ELANES], DT)
                            b2 = 64 * (tk % 2)
                            nc.tensor.matmul(P[:], w["wWX"][b2:b2 + 21, :],
                                             _cols(xfp[b2:b2 + 21, :],
                                                   tk // 2, LE // 2, ELANES),
                                             start=True, stop=False)
                            nc.tensor.matmul(P[:], w["wW1"][:], S128[:],
                                             start=False, stop=True)
                        if r_on:
                            # record hf+hp of step tk-2: S128 B-rows still
                            # hold that state until this tick's cpB
                            sR = tk - 2
                            PR = p_epr.tile([32, ELANES], DT)
                            nc.tensor.matmul(PR[:], w["wSel"][:], S128[:],
                                             start=True, stop=True)
                            for q in range(4):
                                nc.vector.tensor_copy(
                                    _cols(HQR[32 * q:32 * q + 32, :],
                                          sR, LE, NLQ),
                                    PR[:, lq[q] + 1:lq[q] + NLQ + 1])
                        if a_on or b_on:
                            T1 = p_est.tile([128, ELANES], DT)
                            nc.scalar.activation(T1[:], P[:], AF.Tanh)
                            # combined A+B select; B-half mask is staggered
                            # one step back at upload.  At tk=LE this
                            # writes garbage into dead A-state (1-sweep).
                            assert E_SWEEPS == 1
                            nc.vector.copy_predicated(
                                S128[:], mskab(tk), T1[:])
                        if b_on:
                            nc.scalar.activation(S128[64:128, :],
                                                 S128[64:128, :], AF.Gelu)
                        for jw in range(JWARM):
                            PJ = p_epj.tile([128, 128], DT)
                            nc.tensor.matmul(PJ[:], w["wW1"][:],
                                             xfp[:, 128 * jw:128 * jw + 128],
                                             start=True, stop=True)
            if DEBUG_OUTS:
                nc.sync.dma_start(dout["o_hq"][:], HQR[:])

            # ---------------- phase G ----------------
            with tc.tile_pool(name="gat", bufs=1) as p_g:
                HFPK = p_g.tile([128, QWB], DT)
                nc.vector.tensor_copy(wrm[:, 4:5], HQR[:, 0:1])
                NIC = 352  # indirect_copy output-width ISA cap is ~448
                for o in range(0, QWB, NIC):
                    nc.gpsimd.indirect_copy(
                        HFPK[:, o:o + NIC], HQR[:],
                        w["u_gidx"][:, o // 16:(o + NIC) // 16], True)
                nc.vector.tensor_add(RESK[:], HFPK[:], w["u_xsK"][:])
        if DEBUG_OUTS:
            nc.sync.dma_start(dout["o_resk"][:], RESK[:])

        # ---------------- phase C ----------------
        S_C = po.tile([128, QLANES], DT)
        nc.vector.memset(S_C[:], 0.0)
        SHC = po.tile([128, QLANES], DT)
        with tc.tile_pool(name="cst", bufs=3) as p_cst, \
             tc.tile_pool(name="c_ps", bufs=3, space="PSUM") as p_cpp:
            for sweep in range(C_SWEEPS):
                if sweep:
                    nc.vector.tensor_copy(SHC[:], S_C[:])
                    nc.vector.tensor_copy(S_C[:, 1:QLANES],
                                          SHC[:, 0:QLANES - 1])
                    nc.vector.memset(S_C[:, 0:1], 0.0)
                for s in range(LK):
                    P = p_cpp.tile([128, QLANES], DT)
                    nc.tensor.matmul(P[:], w["wC2"][:],
                                     _cols(RESK[:], s, LK, QLANES),
                                     start=True, stop=False)
                    nc.tensor.matmul(P[:], w["wC1"][:], S_C[:],
                                     start=False, stop=True)
                    TC = p_cst.tile([128, QLANES], DT)
                    nc.scalar.activation(TC[:], P[:], AF.Tanh,
                                         bias=bias[:, 0:1])
                    nc.scalar.activation(S_C[:], TC[:], AF.Gelu)
                    if sweep == C_SWEEPS - 1:
                        nc.vector.tensor_copy(_cols(ysK[:], s, LK, NRQ),
                                              S_C[:, 1:QLANES])
        if DEBUG_OUTS:
            nc.sync.dma_start(dout["o_ys"][:], ysK[:])

        # ---------------- phase Z ----------------
        SL = [(i, min(512, QW - i)) for i in range(0, QW, 512)]
        with tc.tile_pool(name="zbuf", bufs=1) as p_z, \
             tc.tile_pool(name="z_ps", bufs=4, space="PSUM") as p_zpp:
            MEAN = p_z.tile([128, QW], DT)
            LV = p_z.tile([128, QW], DT)
            ET = p_z.tile([128, QW], DT)
            PZ = p_z.tile([128, QW], DT)
            ZZ = p_z.tile([128, QW], DT)
            TS = p_z.tile([128, QW], DT)
            GS = p_z.tile([128, QW], DT)
            HSN = p_z.tile([128, QW], DT)
            HPN = p_z.tile([128, QW], DT)

            for dst, wm in ((MEAN, "wMm"), (LV, "wMlv")):
                for o, n in SL:
                    PM = p_zpp.tile([128, 512], DT)
                    nc.tensor.matmul(PM[:, 0:n], w[wm][:], ysK[:, o:o + n],
                                     start=True, stop=True)
                    nc.vector.tensor_copy(dst[:, o:o + n], PM[:, 0:n])
            nc.sync.dma_start(dout["o_mean"][:], MEAN[96:128, :])
            nc.sync.dma_start(dout["o_lv"][:], LV[96:128, :])
            nc.scalar.activation(ET[:], LV[:], AF.Exp,
                                 bias=bias[:, 2:3], scale=0.5)
            nc.vector.tensor_mul(PZ[:], ET[:], w["u_eps"][:])
            nc.vector.scalar_tensor_tensor(
                ZZ[:], MEAN[:], bias[:, 1:2], PZ[:],
                op0=mybir.AluOpType.add, op1=mybir.AluOpType.add)
            for o, n in SL:
                PM = p_zpp.tile([128, 512], DT)
                nc.tensor.matmul(PM[:, 0:n], w["wZ1"][:], ZZ[:, o:o + n],
                                 start=True, stop=True)
                nc.scalar.activation(TS[:, o:o + n], PM[:, 0:n], AF.Tanh,
                                     bias=bias[:, 0:1])
            nc.scalar.activation(GS[:], TS[:], AF.Gelu)
            nc.vector.tensor_mul(HSN[:], GS[:], w["u_mcd"][:])
            for o, n in SL:
                PM = p_zpp.tile([128, 512], DT)
                nc.tensor.matmul(PM[:, 0:n], w["wZ2"][:], HSN[:, o:o + n],
                                 start=True, stop=True)
                nc.scalar.activation(TS[:, o:o + n], PM[:, 0:n], AF.Tanh,
                                     bias=bias[:, 3:4])
            nc.scalar.activation(GS[:], TS[:], AF.Gelu)
            nc.vector.tensor_mul(HPN[:], GS[:], w["u_mcd"][:])
            for o, n in SL:
                PU = p_zpp.tile([128, 512], DT)
                nc.tensor.matmul(PU[0:52, 0:n], w["wU"][:], HPN[:, o:o + n],
                                 start=True, stop=True)
                nc.scalar.activation(UV52[:, o:o + n], PU[0:52, 0:n],
                                     AF.Identity, bias=bias[0:52, 4:5])
            CM = p_z.tile([52, QW], DT)
            CUV = p_z.tile([52, QW], DT)
            nc.vector.tensor_scalar(CM[:], w["u_cnd"][:], -1.0, 1.0,
                                    op0=mybir.AluOpType.mult,
                                    op1=mybir.AluOpType.add)
            nc.vector.tensor_mul(CUV[:], UV52[:], w["u_cnd"][:])
            nc.vector.memset(UVD[:, 0:DPAD], 0.0)
            nc.vector.memset(UVD[:, DPAD + QW:], 0.0)
            nc.vector.tensor_tensor_scan(
                UVD[:, DPAD:DPAD + QW], CM[:], CUV[:], 0.0,
                op0=mybir.AluOpType.mult, op1=mybir.AluOpType.add)
        if DEBUG_OUTS:
            nc.sync.dma_start(dout["o_uvd"][:], UVD[:])

        # ---------------- phase D ----------------
        S_D = po.tile([52, QLANES], DT)
        nc.vector.memset(S_D[:], 0.0)
        SHD = po.tile([52, QLANES], DT)
        with tc.tile_pool(name="dst", bufs=3) as p_dst, \
             tc.tile_pool(name="d_ps", bufs=3, space="PSUM") as p_dpp:
            for sweep in range(D_SWEEPS):
                if sweep:
                    nc.vector.tensor_copy(SHD[:], S_D[:])
                    nc.vector.tensor_copy(S_D[:, 1:QLANES],
                                          SHD[:, 0:QLANES - 1])
                    nc.vector.memset(S_D[:, 0:1], 0.0)
                for s in range(LK):
                    P = p_dpp.tile([52, QLANES], DT)
                    nc.tensor.matmul(P[:], w["wEye"][:],
                                     _cols(UVD[:], s, LK, QLANES),
                                     start=True, stop=False)
                    nc.tensor.matmul(P[:], w["wD"][:], S_D[:],
                                     start=False, stop=True)
                    TD = p_dst.tile([52, QLANES], DT)
                    nc.scalar.activation(TD[:], P[:], AF.Tanh)
                    nc.vector.copy_predicated(
                        S_D[:], _cols(w["u_mcc"][:], s, LK, QLANES), TD[:])
        nc.sync.dma_start(dout["o_SD"][:], S_D[:])

    nc.finalize()
    return nc


# ---------------- entry point ----------------

def kernel(**inputs):
    host = _prep(inputs)
    nc = build_program(host["lq"], host["NLQ"], host["EQ"])

    in_maps = []
    for c in range(NCORES):
        m = {k: np.ascontiguousarray(v) for k, v in host["weights"].items()}
        for name, v in host["cores"][c].items():
            m[name] = np.ascontiguousarray(v)
        in_maps.append(m)

    from concourse.bass_utils import run_bass_kernel_spmd
    res = run_bass_kernel_spmd(nc, in_maps, list(range(NCORES)),
                               trace=TRACE)
    LAST["exec_time_ns"] = res.exec_time_ns
    LAST["results"] = res.results
    LAST["insts"] = res.instructions_and_trace

    K = host["K"]
    g7 = host["geo"][NCORES - 1]
    qs3 = g7["qs"][3]
    r = res.results[NCORES - 1]
    lane = (K - 1 - qs3) // LK + 1
    SD = np.asarray(r["o_SD"])
    hff = np.array([SD[39, lane]], np.float32)
    hfc = SD[40:52, lane].astype(np.float32)
    j = K - 1 - qs3 + EXT
    mean = np.asarray(r["o_mean"])[:, j] + host["bm"]
    lv = np.asarray(r["o_lv"])[:, j] + host["blv"]
    kl = np.float32(-0.5 * np.sum(1.0 + lv - mean * mean - np.exp(lv)))
    return hff, hfc, kl
